# revision 1
# baseline (speedup 1.0000x reference)
"""LLaMA causal self-attention (GQA) on 8 Trainium2 NeuronCores.

Sharding: 2-way data-parallel over batch x 4-way tensor-parallel over KV
groups. Core cid handles batch b=cid//4 and KV group g=cid%4 (q heads
4g..4g+3, kv head g). Each core computes a partial output y_partial =
att_heads @ Wo_rows; the host sums the 4 partials per batch and adds bo.

Per-core pipeline (all layouts chosen so matmul contraction is on the
partition dim and softmax needs no transposes):
  P1: QKV projection (x^T chunks stationary), bias via K=1 ones-row
      matmul, RMSNorm (ACT Square+accum, Newton-refined rsqrt), RoPE
      (elementwise, with qn_w/kn_w folded into the trig tables), then
      PE-transpose q/k to [d, t] layout.
  P2: attention computed transposed: scoresT[k, q] = kT_j^T @ qT chunks,
      additive causal mask on diagonal blocks, exp without max
      subtraction (RMS-normed scores are bounded, softmax is shift
      invariant), softmax denominators via an all-ones stationary matmul
      (sums land broadcast across all partitions), PV accumulated in
      PSUM over j, one normalize multiply per (head, half).
  P3: output projection from attT chunks, PSUM -> SBUF -> DRAM.
"""

import os
from contextlib import ExitStack

import numpy as np

B, T, C = 2, 2048, 2048
H, KV = 16, 4
D = 128
HQ = H // KV        # q heads per core = 4
TB = T // 128       # 16
CB = C // 128       # 16
EPS = 1e-5
SCALE = float(np.float32(1.0) / np.sqrt(np.float32(D)))

_CACHE = {}


def _build(dt_name):
    import concourse.bass as bass
    import concourse.bacc as bacc
    from concourse import mybir
    from concourse.tile import TileContext

    DT = getattr(mybir.dt, dt_name)
    F32 = mybir.dt.float32
    AF = mybir.ActivationFunctionType
    ALU = mybir.AluOpType

    nc = bacc.Bacc(None, target_bir_lowering=False)
    xt = nc.dram_tensor("xt", [TB, 128, CB * 128], DT, kind="ExternalInput")
    wqkv = nc.dram_tensor("wqkv", [CB, 128, 768], DT, kind="ExternalInput")
    bqkv = nc.dram_tensor("bqkv", [1, 768], DT, kind="ExternalInput")
    trig = nc.dram_tensor("trig", [TB, 128, 8 * 64], DT, kind="ExternalInput")
    maskt = nc.dram_tensor("maskt", [128, 128], F32, kind="ExternalInput")
    cst = nc.dram_tensor("cst", [2, 128, 128], DT, kind="ExternalInput")
    wo = nc.dram_tensor("wo", [HQ, 128, C], DT, kind="ExternalInput")
    y = nc.dram_tensor("y", [T, C], F32, kind="ExternalOutput")

    def bc4(apv, n):
        # broadcast a [128, 64] AP along a new middle (head) dim of size n
        return bass.AP(tensor=apv.tensor, offset=apv.offset,
                       ap=[list(apv.ap[0]), [0, n], [1, 64]])

    with TileContext(nc) as tc, ExitStack() as ctx:
        persist = ctx.enter_context(tc.tile_pool(name="persist", bufs=1))
        ones = persist.tile([128, 128], DT)
        ident = persist.tile([128, 128], DT)
        nc.sync.dma_start(out=ones, in_=cst[0])
        nc.sync.dma_start(out=ident, in_=cst[1])
        maskt_sb = persist.tile([128, 128], F32)
        nc.sync.dma_start(out=maskt_sb, in_=maskt[:, :])
        bq_sb = persist.tile([1, 768], DT)
        nc.sync.dma_start(out=bq_sb, in_=bqkv[:, :])
        # [d, seg, t]: segs 0..3 = q heads, seg 4 = k
        qkT = persist.tile([128, 5, T], DT)
        vbuf = persist.tile([128, TB, 128], DT)   # [t-in-block, j, d]
        attT = persist.tile([128, HQ, T], DT)     # [d, head, t]
        # ---------------- P1: QKV + RMSNorm + RoPE + transpose ----------
        with tc.tile_pool(name="p1", bufs=2) as p1, \
             tc.tile_pool(name="p1x", bufs=2) as p1x, \
             tc.tile_pool(name="p1s", bufs=3) as p1s, \
             tc.tile_pool(name="p1w", bufs=1) as p1w, \
             tc.tile_pool(name="p1ps", bufs=2, space="PSUM") as p1ps, \
             tc.tile_pool(name="p1tp", bufs=2, space="PSUM") as p1tp:
            wqkv_sb = p1w.tile([128, CB, 768], DT)
            for cc in range(CB):
                nc.sync.dma_start(out=wqkv_sb[:, cc, :], in_=wqkv[cc])
            for tt in range(TB):
                xtall = p1x.tile([128, CB, 128], DT, tag="xt")
                nc.sync.dma_start(out=xtall, in_=xt[tt])
                trig_sb = p1x.tile([128, 8, 64], DT, tag="trig")
                nc.sync.dma_start(out=trig_sb, in_=trig[tt])

                qkv_ps = p1ps.tile([128, 768], F32, tag="qkv")
                for cc in range(CB):
                    nc.tensor.matmul(qkv_ps[:, 0:512], xtall[:, cc, :],
                                     wqkv_sb[:, cc, 0:512],
                                     start=(cc == 0), stop=False)
                    nc.tensor.matmul(qkv_ps[:, 512:768], xtall[:, cc, :],
                                     wqkv_sb[:, cc, 512:768],
                                     start=(cc == 0), stop=False)
                nc.tensor.matmul(qkv_ps[:, 0:512], ones[0:1, :],
                                 bq_sb[0:1, 0:512], start=False, stop=True)
                nc.tensor.matmul(qkv_ps[:, 512:768], ones[0:1, :],
                                 bq_sb[0:1, 512:768], start=False, stop=True)

                # RMSNorm stats for 4 q heads + k
                ssq = p1s.tile([128, 8], F32, tag="ssq")
                for s in range(5):
                    sqs = p1s.tile([128, 128], F32, tag="sqs")
                    nc.scalar.activation(out=sqs, in_=qkv_ps[:, s * 128:(s + 1) * 128],
                                         func=AF.Square, accum_out=ssq[:, s:s + 1])
                x5 = p1s.tile([128, 8], F32, tag="x5")
                nc.scalar.activation(out=x5[:, 0:5], in_=ssq[:, 0:5],
                                     func=AF.Copy, scale=1.0 / D, bias=EPS)
                sq5 = p1s.tile([128, 8], F32, tag="sq5")
                nc.scalar.activation(out=sq5[:, 0:5], in_=x5[:, 0:5], func=AF.Sqrt)
                r0 = p1s.tile([128, 8], F32, tag="r0")
                nc.vector.reciprocal(out=r0[:, 0:5], in_=sq5[:, 0:5])
                # one Newton step: rstd = r0 * (1.5 - 0.5 * x * r0^2)
                t1 = p1s.tile([128, 8], F32, tag="t1")
                nc.vector.tensor_mul(t1[:, 0:5], r0[:, 0:5], r0[:, 0:5])
                t2 = p1s.tile([128, 8], F32, tag="t2")
                nc.vector.scalar_tensor_tensor(out=t2[:, 0:5], in0=t1[:, 0:5],
                                               scalar=-0.5, in1=x5[:, 0:5],
                                               op0=ALU.mult, op1=ALU.mult)
                t3 = p1s.tile([128, 8], F32, tag="t3")
                nc.vector.tensor_scalar_add(t3[:, 0:5], t2[:, 0:5], 1.5)
                rstd = p1s.tile([128, 8], F32, tag="rstd")
                nc.vector.tensor_mul(rstd[:, 0:5], r0[:, 0:5], t3[:, 0:5])

                # normalize q/k (x * rstd), copy v
                qn = p1.tile([128, 640], F32, tag="qn")
                for s in range(5):
                    nc.scalar.activation(out=qn[:, s * 128:(s + 1) * 128],
                                         in_=qkv_ps[:, s * 128:(s + 1) * 128],
                                         func=AF.Copy, scale=rstd[:, s:s + 1], bias=0.0)
                nc.scalar.activation(out=vbuf[:, tt, :], in_=qkv_ps[:, 640:768],
                                     func=AF.Copy, scale=1.0, bias=0.0)

                # RoPE (norm weights folded into trig tables host-side)
                qr = p1.tile([128, 640], DT, tag="qr")

                def rope(seg0, nseg, toff):
                    src = qn[:, seg0 * 128:(seg0 + nseg) * 128]
                    dst = qr[:, seg0 * 128:(seg0 + nseg) * 128]
                    sev = src.rearrange("p (h j t) -> p h t j", t=2, j=64)
                    dev = dst.rearrange("p (h j t) -> p h t j", t=2, j=64)
                    qe, qo = sev[:, :, 0, :], sev[:, :, 1, :]
                    re, ro = dev[:, :, 0, :], dev[:, :, 1, :]
                    ce = bc4(trig_sb[:, toff + 0, :], nseg)
                    so = bc4(trig_sb[:, toff + 1, :], nseg)
                    se = bc4(trig_sb[:, toff + 2, :], nseg)
                    co = bc4(trig_sb[:, toff + 3, :], nseg)
                    ta = p1s.tile([128, nseg, 64], F32, tag=f"ra{toff}")
                    tb = p1s.tile([128, nseg, 64], F32, tag=f"rb{toff}")
                    nc.vector.tensor_mul(ta, qe, ce)
                    nc.gpsimd.tensor_mul(tb, qo, so)
                    nc.vector.tensor_sub(re, ta, tb)
                    tc_ = p1s.tile([128, nseg, 64], F32, tag=f"rc{toff}")
                    td = p1s.tile([128, nseg, 64], F32, tag=f"rd{toff}")
                    nc.gpsimd.tensor_mul(tc_, qe, se)
                    nc.vector.tensor_mul(td, qo, co)
                    nc.gpsimd.tensor_add(ro, tc_, td)

                rope(0, 4, 0)   # q heads, tables 0..3
                rope(4, 1, 4)   # k, tables 4..7

                # transpose to [d, t]
                for s in range(5):
                    tp = p1tp.tile([128, 128], DT, tag="tp")
                    nc.tensor.transpose(tp, qr[:, s * 128:(s + 1) * 128], ident)
                    nc.scalar.activation(out=qkT[:, s, tt * 128:(tt + 1) * 128],
                                         in_=tp, func=AF.Copy, scale=1.0, bias=0.0)

        # ---------------- P2: attention (transposed scores) -------------
        with tc.tile_pool(name="p2", bufs=2) as p2, \
             tc.tile_pool(name="p2acc", bufs=1, space="PSUM") as p2acc, \
             tc.tile_pool(name="p2sc", bufs=2, space="PSUM") as p2sc:
            for h in range(HQ):
                for half in range(2):
                    q0 = half * 1024
                    q1 = q0 + 1024
                    jmax = 7 if half == 0 else 15
                    outT = p2acc.tile([128, 1024], F32, tag="outT")
                    sums = p2acc.tile([128, 1024], F32, tag="sums")
                    for j in range(jmax + 1):
                        qlo = max(q0, j * 128)
                        pT = p2.tile([128, 1024], DT, tag="pT")
                        kTj = qkT[:, 4, j * 128:(j + 1) * 128]
                        for ci in range(2):
                            begin = max(qlo, q0 + 512 * ci)
                            end = q0 + 512 * (ci + 1)
                            if begin >= end:
                                continue
                            w = end - begin
                            sc = p2sc.tile([128, 512], F32, tag="sc")
                            nc.tensor.matmul(sc[:, 0:w], kTj,
                                             qkT[:, h, begin:end],
                                             start=True, stop=True)
                            if begin == j * 128:
                                nc.vector.tensor_add(sc[:, 0:128],
                                                     sc[:, 0:128], maskt_sb)
                            poff = begin - qlo
                            nc.scalar.activation(out=pT[:, poff:poff + w],
                                                 in_=sc[:, 0:w], func=AF.Exp,
                                                 scale=SCALE)
                            nc.tensor.matmul(outT[:, begin - q0:begin - q0 + w],
                                             vbuf[:, j, :], pT[:, poff:poff + w],
                                             start=(j == 0), stop=(j == jmax),
                                             skip_group_check=True)
                            nc.tensor.matmul(sums[:, begin - q0:begin - q0 + w],
                                             ones, pT[:, poff:poff + w],
                                             start=(j == 0), stop=(j == jmax),
                                             skip_group_check=True)
                    inv = p2.tile([128, 1024], F32, tag="inv")
                    nc.vector.reciprocal(out=inv, in_=sums)
                    nc.vector.tensor_mul(attT[:, h, q0:q1], outT, inv)

        # ---------------- P3: output projection --------------------------
        with tc.tile_pool(name="p3", bufs=2) as p3, \
             tc.tile_pool(name="p3w", bufs=1) as p3w, \
             tc.tile_pool(name="p3ps", bufs=2, space="PSUM") as p3ps:
            wo_sb = p3w.tile([128, HQ, C], DT)
            for h in range(HQ):
                nc.sync.dma_start(out=wo_sb[:, h, :], in_=wo[h])
            for tt in range(TB):
                y_ps = p3ps.tile([128, 2048], F32, tag="y")
                for h in range(HQ):
                    for c4 in range(4):
                        nc.tensor.matmul(y_ps[:, c4 * 512:(c4 + 1) * 512],
                                         attT[:, h, tt * 128:(tt + 1) * 128],
                                         wo_sb[:, h, c4 * 512:(c4 + 1) * 512],
                                         start=(h == 0), stop=(h == HQ - 1))
                y_sb = p3.tile([128, 2048], F32, tag="ysb")
                nc.scalar.activation(out=y_sb[:, 0:1024], in_=y_ps[:, 0:1024],
                                     func=AF.Copy, scale=1.0, bias=0.0)
                nc.vector.tensor_copy(y_sb[:, 1024:2048], y_ps[:, 1024:2048])
                nc.sync.dma_start(out=y[tt * 128:(tt + 1) * 128, :], in_=y_sb)

    nc.compile()
    return nc


def _prep_core_inputs(b, g, x, Wq, bq, Wk, bk, Wv, bv, Wo, bo, qn_w, kn_w,
                      freqs_cos, freqs_sin, mask):
    f32 = np.float32
    xb = np.ascontiguousarray(x[b], dtype=f32)
    # [tt, csub, cc, tcol]: xt[tt][p][cc*128+tc] = x[b][tt*128+tc][cc*128+p]
    xt = np.ascontiguousarray(
        xb.reshape(TB, 128, CB, 128).transpose(0, 3, 2, 1)
    ).reshape(TB, 128, CB * 128)
    wqkv = np.ascontiguousarray(np.concatenate([
        Wq[:, g * 512:(g + 1) * 512],
        Wk[:, g * 128:(g + 1) * 128],
        Wv[:, g * 128:(g + 1) * 128],
    ], axis=1).reshape(CB, 128, 768), dtype=f32)
    bqkv = np.concatenate([
        bq[g * 512:(g + 1) * 512], bk[g * 128:(g + 1) * 128],
        bv[g * 128:(g + 1) * 128],
    ]).reshape(1, 768).astype(f32)
    cos = freqs_cos.astype(f32)
    sin = freqs_sin.astype(f32)
    qe, qo = qn_w[0::2].astype(f32), qn_w[1::2].astype(f32)
    ke, ko = kn_w[0::2].astype(f32), kn_w[1::2].astype(f32)
    # tables: [ce, so, se, co] for q then for k; layout [TB, 128, 8*64]
    tabs = np.stack([cos * qe, sin * qo, sin * qe, cos * qo,
                     cos * ke, sin * ko, sin * ke, cos * ko], axis=1)  # [T, 8, 64]
    trig = np.ascontiguousarray(tabs.reshape(TB, 128, 8 * 64), dtype=f32)
    maskt = np.ascontiguousarray(mask[0, 0, :128, :128].T, dtype=f32)
    cst = np.stack([np.ones((128, 128), f32), np.eye(128, dtype=f32)])
    wo_t = np.ascontiguousarray(
        Wo[g * 512:(g + 1) * 512].reshape(HQ, 128, C), dtype=f32)
    return {"xt": xt, "wqkv": wqkv, "bqkv": bqkv, "trig": trig,
            "maskt": maskt, "cst": cst, "wo": wo_t}


def kernel(x, Wq, bq, Wk, bk, Wv, bv, Wo, bo, qn_w, kn_w,
           freqs_cos, freqs_sin, mask, _trace=False, _trace_kwargs=None):
    from concourse.bass_utils import run_bass_kernel_spmd

    args = (np.asarray(x), np.asarray(Wq), np.asarray(bq), np.asarray(Wk),
            np.asarray(bk), np.asarray(Wv), np.asarray(bv), np.asarray(Wo),
            np.asarray(bo), np.asarray(qn_w), np.asarray(kn_w),
            np.asarray(freqs_cos), np.asarray(freqs_sin), np.asarray(mask))
    bo_np = args[8].astype(np.float32)

    dt_name = os.environ.get("BASS_ATTN_DT", "float32r")
    if dt_name not in _CACHE:
        _CACHE[dt_name] = _build(dt_name)
    nc = _CACHE[dt_name]

    in_maps = [_prep_core_inputs(cid // 4, cid % 4, *args) for cid in range(8)]
    res = run_bass_kernel_spmd(nc, in_maps, core_ids=list(range(8)),
                               trace=_trace, **(_trace_kwargs or {}))
    outs = [res.results[i]["y"] for i in range(8)]
    yfull = np.empty((B, T, C), dtype=np.float32)
    for b in range(B):
        yfull[b] = outs[4 * b] + outs[4 * b + 1] + outs[4 * b + 2] + outs[4 * b + 3]
        yfull[b] += bo_np[None, :]
    if _trace:
        kernel._last_result = res
    return yfull



# revision 5
# speedup vs baseline: 1.0915x; 1.0915x over previous
"""LLaMA causal self-attention (GQA) on 8 Trainium2 NeuronCores.

Sharding: 2-way data-parallel over batch x 4-way tensor-parallel over KV
groups. Core cid handles batch b=cid//4 and KV group g=cid%4 (q heads
4g..4g+3, kv head g). Each core computes a partial output y_partial =
att_heads @ Wo_rows; the host sums the 4 partials per batch and adds bo.

Per-core pipeline (all layouts chosen so matmul contraction is on the
partition dim and softmax needs no transposes):
  P1: QKV projection (x^T chunks stationary), bias via K=1 ones-row
      matmul, RMSNorm (ACT Square+accum, Newton-refined rsqrt), RoPE
      (elementwise, with qn_w/kn_w folded into the trig tables), then
      PE-transpose q/k to [d, t] layout.
  P2: attention computed transposed: scoresT[k, q] = kT_j^T @ qT chunks,
      additive causal mask on diagonal blocks, exp without max
      subtraction (RMS-normed scores are bounded, softmax is shift
      invariant), softmax denominators via an all-ones stationary matmul
      (sums land broadcast across all partitions), PV accumulated in
      PSUM over j, one normalize multiply per (head, half).
  P3: output projection from attT chunks, PSUM -> SBUF -> DRAM.
"""

import os
from contextlib import ExitStack

import numpy as np

B, T, C = 2, 2048, 2048
H, KV = 16, 4
D = 128
HQ = H // KV        # q heads per core = 4
TB = T // 128       # 16
CB = C // 128       # 16
EPS = 1e-5
SCALE = float(np.float32(1.0) / np.sqrt(np.float32(D)))

_CACHE = {}


def _build(dt_name):
    import concourse.bass as bass
    import concourse.bacc as bacc
    from concourse import mybir
    from concourse.tile import TileContext

    DT = getattr(mybir.dt, dt_name)
    F32 = mybir.dt.float32
    AF = mybir.ActivationFunctionType
    ALU = mybir.AluOpType

    nc = bacc.Bacc(None, target_bir_lowering=False)
    xt = nc.dram_tensor("xt", [TB, 128, CB * 128], DT, kind="ExternalInput")
    wqkv = nc.dram_tensor("wqkv", [CB, 128, 768], DT, kind="ExternalInput")
    bqkv = nc.dram_tensor("bqkv", [1, 768], DT, kind="ExternalInput")
    trig = nc.dram_tensor("trig", [TB, 128, 8 * 64], DT, kind="ExternalInput")
    maskt = nc.dram_tensor("maskt", [128, 128], F32, kind="ExternalInput")
    cst = nc.dram_tensor("cst", [2, 128, 128], DT, kind="ExternalInput")
    wo = nc.dram_tensor("wo", [HQ, 128, C], DT, kind="ExternalInput")
    y = nc.dram_tensor("y", [T, C], F32, kind="ExternalOutput")

    def bc4(apv, n):
        # broadcast a [128, 64] AP along a new middle (head) dim of size n
        return bass.AP(tensor=apv.tensor, offset=apv.offset,
                       ap=[list(apv.ap[0]), [0, n], [1, 64]])

    with TileContext(nc) as tc, ExitStack() as ctx:
        persist = ctx.enter_context(tc.tile_pool(name="persist", bufs=1))
        ones = persist.tile([128, 128], DT)
        ident = persist.tile([128, 128], DT)
        nc.sync.dma_start(out=ones, in_=cst[0])
        nc.sync.dma_start(out=ident, in_=cst[1])
        maskt_sb = persist.tile([128, 128], F32)
        nc.sync.dma_start(out=maskt_sb, in_=maskt[:, :])
        bq_sb = persist.tile([1, 768], DT)
        nc.sync.dma_start(out=bq_sb, in_=bqkv[:, :])
        # [d, seg, t]: segs 0..3 = q heads, seg 4 = k
        qkT = persist.tile([128, 5, T], DT)
        vbuf = persist.tile([128, TB, 128], DT)   # [t-in-block, j, d]
        attT = persist.tile([128, HQ, T], DT)     # [d, head, t]
        # ---------------- P1: QKV + RMSNorm + RoPE + transpose ----------
        with tc.tile_pool(name="p1", bufs=2) as p1, \
             tc.tile_pool(name="p1x", bufs=2) as p1x, \
             tc.tile_pool(name="p1s", bufs=3) as p1s, \
             tc.tile_pool(name="p1w", bufs=1) as p1w, \
             tc.tile_pool(name="p1ps", bufs=2, space="PSUM") as p1ps, \
             tc.tile_pool(name="p1tp", bufs=2, space="PSUM") as p1tp:
            wqkv_sb = p1w.tile([128, CB, 768], DT)
            for cc in range(CB):
                nc.sync.dma_start(out=wqkv_sb[:, cc, :], in_=wqkv[cc])
            for tt in range(TB):
                xtall = p1x.tile([128, CB, 128], DT, tag="xt")
                nc.sync.dma_start(out=xtall, in_=xt[tt])
                trig_sb = p1x.tile([128, 8, 64], DT, tag="trig")
                nc.sync.dma_start(out=trig_sb, in_=trig[tt])

                qkv_ps = p1ps.tile([128, 768], F32, tag="qkv")
                for cc in range(CB):
                    nc.tensor.matmul(qkv_ps[:, 0:512], xtall[:, cc, :],
                                     wqkv_sb[:, cc, 0:512],
                                     start=(cc == 0), stop=False)
                    nc.tensor.matmul(qkv_ps[:, 512:768], xtall[:, cc, :],
                                     wqkv_sb[:, cc, 512:768],
                                     start=(cc == 0), stop=False)
                nc.tensor.matmul(qkv_ps[:, 0:512], ones[0:1, :],
                                 bq_sb[0:1, 0:512], start=False, stop=True)
                nc.tensor.matmul(qkv_ps[:, 512:768], ones[0:1, :],
                                 bq_sb[0:1, 512:768], start=False, stop=True)

                # RMSNorm stats for 4 q heads + k
                ssq = p1s.tile([128, 8], F32, tag="ssq")
                for s in range(5):
                    sqs = p1s.tile([128, 128], F32, tag="sqs")
                    nc.scalar.activation(out=sqs, in_=qkv_ps[:, s * 128:(s + 1) * 128],
                                         func=AF.Square, accum_out=ssq[:, s:s + 1])
                x5 = p1s.tile([128, 8], F32, tag="x5")
                nc.scalar.activation(out=x5[:, 0:5], in_=ssq[:, 0:5],
                                     func=AF.Copy, scale=1.0 / D, bias=EPS)
                sq5 = p1s.tile([128, 8], F32, tag="sq5")
                nc.scalar.activation(out=sq5[:, 0:5], in_=x5[:, 0:5], func=AF.Sqrt)
                r0 = p1s.tile([128, 8], F32, tag="r0")
                nc.vector.reciprocal(out=r0[:, 0:5], in_=sq5[:, 0:5])
                # one Newton step: rstd = r0 * (1.5 - 0.5 * x * r0^2)
                t1 = p1s.tile([128, 8], F32, tag="t1")
                nc.vector.tensor_mul(t1[:, 0:5], r0[:, 0:5], r0[:, 0:5])
                t2 = p1s.tile([128, 8], F32, tag="t2")
                nc.vector.scalar_tensor_tensor(out=t2[:, 0:5], in0=t1[:, 0:5],
                                               scalar=-0.5, in1=x5[:, 0:5],
                                               op0=ALU.mult, op1=ALU.mult)
                t3 = p1s.tile([128, 8], F32, tag="t3")
                nc.vector.tensor_scalar_add(t3[:, 0:5], t2[:, 0:5], 1.5)
                rstd = p1s.tile([128, 8], F32, tag="rstd")
                nc.vector.tensor_mul(rstd[:, 0:5], r0[:, 0:5], t3[:, 0:5])

                # normalize q/k (x * rstd), copy v
                qn = p1.tile([128, 640], F32, tag="qn")
                for s in range(5):
                    nc.scalar.activation(out=qn[:, s * 128:(s + 1) * 128],
                                         in_=qkv_ps[:, s * 128:(s + 1) * 128],
                                         func=AF.Copy, scale=rstd[:, s:s + 1], bias=0.0)
                nc.scalar.activation(out=vbuf[:, tt, :], in_=qkv_ps[:, 640:768],
                                     func=AF.Copy, scale=1.0, bias=0.0)

                # RoPE (norm weights folded into trig tables host-side)
                qr = p1.tile([128, 640], DT, tag="qr")

                def rope(seg0, nseg, toff):
                    src = qn[:, seg0 * 128:(seg0 + nseg) * 128]
                    dst = qr[:, seg0 * 128:(seg0 + nseg) * 128]
                    sev = src.rearrange("p (h j t) -> p h t j", t=2, j=64)
                    dev = dst.rearrange("p (h j t) -> p h t j", t=2, j=64)
                    qe, qo = sev[:, :, 0, :], sev[:, :, 1, :]
                    re, ro = dev[:, :, 0, :], dev[:, :, 1, :]
                    ce = bc4(trig_sb[:, toff + 0, :], nseg)
                    so = bc4(trig_sb[:, toff + 1, :], nseg)
                    se = bc4(trig_sb[:, toff + 2, :], nseg)
                    co = bc4(trig_sb[:, toff + 3, :], nseg)
                    ta = p1s.tile([128, nseg, 64], F32, tag=f"ra{toff}")
                    tb = p1s.tile([128, nseg, 64], F32, tag=f"rb{toff}")
                    nc.vector.tensor_mul(ta, qe, ce)
                    nc.gpsimd.tensor_mul(tb, qo, so)
                    nc.vector.tensor_sub(re, ta, tb)
                    tc_ = p1s.tile([128, nseg, 64], F32, tag=f"rc{toff}")
                    td = p1s.tile([128, nseg, 64], F32, tag=f"rd{toff}")
                    nc.gpsimd.tensor_mul(tc_, qe, se)
                    nc.vector.tensor_mul(td, qo, co)
                    nc.gpsimd.tensor_add(ro, tc_, td)

                rope(0, 4, 0)   # q heads, tables 0..3
                rope(4, 1, 4)   # k, tables 4..7

                # transpose to [d, t]
                for s in range(5):
                    tp = p1tp.tile([128, 128], DT, tag="tp")
                    nc.tensor.transpose(tp, qr[:, s * 128:(s + 1) * 128], ident)
                    nc.scalar.activation(out=qkT[:, s, tt * 128:(tt + 1) * 128],
                                         in_=tp, func=AF.Copy, scale=1.0, bias=0.0)

        # ---------------- P2: attention (transposed scores) -------------
        with tc.tile_pool(name="p2", bufs=2) as p2, \
             tc.tile_pool(name="p2acc", bufs=1, space="PSUM") as p2acc, \
             tc.tile_pool(name="p2sc", bufs=2, space="PSUM") as p2sc:
            for h in range(HQ):
                for half in range(2):
                    q0 = half * 1024
                    q1 = q0 + 1024
                    jmax = 7 if half == 0 else 15
                    outT = p2acc.tile([128, 1024], F32, tag="outT")
                    sums = p2acc.tile([128, 1024], F32, tag="sums")
                    for j in range(jmax + 1):
                        qlo = max(q0, j * 128)
                        pT = p2.tile([128, 1024], DT, tag="pT")
                        kTj = qkT[:, 4, j * 128:(j + 1) * 128]
                        for ci in range(2):
                            begin = max(qlo, q0 + 512 * ci)
                            end = q0 + 512 * (ci + 1)
                            if begin >= end:
                                continue
                            w = end - begin
                            sc = p2sc.tile([128, 512], F32, tag="sc")
                            nc.tensor.matmul(sc[:, 0:w], kTj,
                                             qkT[:, h, begin:end],
                                             start=True, stop=True)
                            if begin == j * 128:
                                nc.vector.tensor_add(sc[:, 0:128],
                                                     sc[:, 0:128], maskt_sb)
                            poff = begin - qlo
                            nc.scalar.activation(out=pT[:, poff:poff + w],
                                                 in_=sc[:, 0:w], func=AF.Exp,
                                                 scale=SCALE)
                            nc.tensor.matmul(outT[:, begin - q0:begin - q0 + w],
                                             vbuf[:, j, :], pT[:, poff:poff + w],
                                             start=(j == 0), stop=(j == jmax),
                                             skip_group_check=True)
                            nc.tensor.matmul(sums[:, begin - q0:begin - q0 + w],
                                             ones, pT[:, poff:poff + w],
                                             start=(j == 0), stop=(j == jmax),
                                             skip_group_check=True)
                    inv = p2.tile([128, 1024], F32, tag="inv")
                    nc.vector.reciprocal(out=inv, in_=sums)
                    nc.vector.tensor_mul(attT[:, h, q0:q1], outT, inv)

        # ---------------- P3: output projection --------------------------
        with tc.tile_pool(name="p3", bufs=2) as p3, \
             tc.tile_pool(name="p3w", bufs=1) as p3w, \
             tc.tile_pool(name="p3ps", bufs=2, space="PSUM") as p3ps:
            wo_sb = p3w.tile([128, HQ, C], DT)
            for h in range(HQ):
                nc.sync.dma_start(out=wo_sb[:, h, :], in_=wo[h])
            for tt in range(TB):
                y_ps = p3ps.tile([128, 2048], F32, tag="y")
                for h in range(HQ):
                    for c4 in range(4):
                        nc.tensor.matmul(y_ps[:, c4 * 512:(c4 + 1) * 512],
                                         attT[:, h, tt * 128:(tt + 1) * 128],
                                         wo_sb[:, h, c4 * 512:(c4 + 1) * 512],
                                         start=(h == 0), stop=(h == HQ - 1))
                y_sb = p3.tile([128, 2048], F32, tag="ysb")
                nc.scalar.activation(out=y_sb[:, 0:1024], in_=y_ps[:, 0:1024],
                                     func=AF.Copy, scale=1.0, bias=0.0)
                nc.vector.tensor_copy(y_sb[:, 1024:2048], y_ps[:, 1024:2048])
                nc.sync.dma_start(out=y[tt * 128:(tt + 1) * 128, :], in_=y_sb)

    nc.compile()
    return nc


def _build_fused(dt_name):
    """Fused software pipeline: P1 (QKV+RMSNorm+RoPE+transpose), P2
    (attention) and P3 (output proj) interleaved over 512-row q-stripes
    so the tensor engine stays dense and warm.  Emission order:
    P1(0) P1(1) [P2(s) P3(s) P1(s+2)] for s=0..3.

    PSUM budget (8 banks): qkv accumulator [128,768] x1 = 2, shared
    work pool [128,512] x4 = 4 (scores / transposes / y chunks),
    outT x1 = 1, sums x1 = 1.
    """
    import concourse.bass as bass
    import concourse.bacc as bacc
    from concourse import mybir
    from concourse.tile import TileContext

    DT = getattr(mybir.dt, dt_name)
    F32 = mybir.dt.float32
    AF = mybir.ActivationFunctionType

    nc = bacc.Bacc(None, target_bir_lowering=False)
    xt = nc.dram_tensor("xt", [TB, 128, CB * 128], DT, kind="ExternalInput")
    wqkv = nc.dram_tensor("wqkv", [CB, 128, 768], DT, kind="ExternalInput")
    bqkv = nc.dram_tensor("bqkv", [1, 768], DT, kind="ExternalInput")
    trig = nc.dram_tensor("trig", [TB, 128, 8 * 64], F32, kind="ExternalInput")
    maskt = nc.dram_tensor("maskt", [128, 128], F32, kind="ExternalInput")
    cst = nc.dram_tensor("cst", [2, 128, 128], DT, kind="ExternalInput")
    wo = nc.dram_tensor("wo", [HQ, 128, C], DT, kind="ExternalInput")
    y = nc.dram_tensor("y", [T, C], DT, kind="ExternalOutput")

    def bc(apv, n, w=64):
        # broadcast a [128, w] AP along a new middle dim of size n
        return bass.AP(tensor=apv.tensor, offset=apv.offset,
                       ap=[list(apv.ap[0]), [0, n], [1, w]])

    with TileContext(nc) as tc, ExitStack() as ctx:
        persist = ctx.enter_context(tc.tile_pool(name="persist", bufs=1))
        p1x = ctx.enter_context(tc.tile_pool(name="p1x", bufs=3))
        p1s = ctx.enter_context(tc.tile_pool(name="p1s", bufs=3))
        p2 = ctx.enter_context(tc.tile_pool(name="p2", bufs=3))
        p3 = ctx.enter_context(tc.tile_pool(name="p3", bufs=3))
        p1ps = ctx.enter_context(tc.tile_pool(name="p1ps", bufs=1, space="PSUM"))
        work = ctx.enter_context(tc.tile_pool(name="work", bufs=4, space="PSUM"))
        outTp = ctx.enter_context(tc.tile_pool(name="outTp", bufs=1, space="PSUM"))
        sumsp = ctx.enter_context(tc.tile_pool(name="sumsp", bufs=1, space="PSUM"))

        ones = persist.tile([128, 128], DT)
        ident = persist.tile([128, 128], DT)
        nc.sync.dma_start(out=ones, in_=cst[0])
        nc.sync.dma_start(out=ident, in_=cst[1])
        maskt_sb = persist.tile([128, 128], F32)
        nc.sync.dma_start(out=maskt_sb, in_=maskt[:, :])
        bq_sb = persist.tile([1, 768], DT)
        nc.sync.dma_start(out=bq_sb, in_=bqkv[:, :])
        # [d, seg, t]: segs 0..3 = q heads, seg 4 = k
        qkT = persist.tile([128, 5, T], DT)
        vbuf = persist.tile([128, TB, 128], DT)   # [t-in-block, j, d]
        attT = persist.tile([128, HQ, T], DT)     # [d, head, t]
        wqkv_sb = persist.tile([128, CB, 768], DT)
        for cc in range(CB):
            nc.sync.dma_start(out=wqkv_sb[:, cc, :], in_=wqkv[cc])
        wo_sb = persist.tile([128, HQ, C], DT)
        for h in range(HQ):
            nc.sync.dma_start(out=wo_sb[:, h, :], in_=wo[h])

        def p1(s):
            for tt in range(4 * s, 4 * s + 4):
                xtall = p1x.tile([128, CB, 128], DT, tag="xt")
                nc.sync.dma_start(out=xtall, in_=xt[tt])
                trig_sb = p1x.tile([128, 8, 64], F32, tag="trig")
                nc.sync.dma_start(out=trig_sb, in_=trig[tt])

                qkv_ps = p1ps.tile([128, 768], F32, tag="qkv")
                for cc in range(CB):
                    nc.tensor.matmul(qkv_ps[:, 0:512], xtall[:, cc, :],
                                     wqkv_sb[:, cc, 0:512],
                                     start=(cc == 0), stop=False)
                    nc.tensor.matmul(qkv_ps[:, 512:768], xtall[:, cc, :],
                                     wqkv_sb[:, cc, 512:768],
                                     start=(cc == 0), stop=False)
                nc.tensor.matmul(qkv_ps[:, 0:512], ones[0:1, :],
                                 bq_sb[0:1, 0:512], start=False, stop=True)
                nc.tensor.matmul(qkv_ps[:, 512:768], ones[0:1, :],
                                 bq_sb[0:1, 512:768], start=False, stop=True)

                # evacuate PSUM fast: q/k raw to SBUF f32, v to vbuf bf16
                qsb = p1s.tile([128, 640], F32, tag="qsb")
                nc.scalar.activation(out=qsb[:, 0:384], in_=qkv_ps[:, 0:384],
                                     func=AF.Copy, scale=1.0, bias=0.0)
                nc.vector.tensor_copy(qsb[:, 384:640], qkv_ps[:, 384:640])
                nc.vector.tensor_copy(vbuf[:, tt, :], qkv_ps[:, 640:768])

                # RMSNorm stats (5 segs: 4 q heads + k)
                ssq = p1s.tile([128, 8], F32, tag="ssq")
                for sg in range(5):
                    sqs = p1s.tile([128, 128], F32, tag="sqs")
                    nc.scalar.activation(out=sqs, in_=qsb[:, sg * 128:(sg + 1) * 128],
                                         func=AF.Square, accum_out=ssq[:, sg:sg + 1])
                x5 = p1s.tile([128, 8], F32, tag="x5")
                nc.scalar.activation(out=x5[:, 0:5], in_=ssq[:, 0:5],
                                     func=AF.Copy, scale=1.0 / D, bias=EPS)
                sq5 = p1s.tile([128, 8], F32, tag="sq5")
                nc.scalar.activation(out=sq5[:, 0:5], in_=x5[:, 0:5], func=AF.Sqrt)
                rstd = p1s.tile([128, 8], F32, tag="rstd")
                nc.vector.reciprocal(out=rstd[:, 0:5], in_=sq5[:, 0:5])

                # RoPE with rstd folded in post-combine; qn_w/kn_w are in trig
                qr = p1s.tile([128, 640], DT, tag="qr")

                def rope(seg0, nseg, toff):
                    src = qsb[:, seg0 * 128:(seg0 + nseg) * 128]
                    dst = qr[:, seg0 * 128:(seg0 + nseg) * 128]
                    sev = src.rearrange("p (h j t) -> p h t j", t=2, j=64)
                    dev = dst.rearrange("p (h j t) -> p h t j", t=2, j=64)
                    qe, qo = sev[:, :, 0, :], sev[:, :, 1, :]
                    re, ro = dev[:, :, 0, :], dev[:, :, 1, :]
                    ce = bc(trig_sb[:, toff + 0, :], nseg)
                    so = bc(trig_sb[:, toff + 1, :], nseg)
                    se = bc(trig_sb[:, toff + 2, :], nseg)
                    co = bc(trig_sb[:, toff + 3, :], nseg)
                    rsb = bass.AP(tensor=rstd.tensor, offset=rstd.offset + seg0,
                                  ap=[list(rstd.ap[0]), [1, nseg], [0, 64]])
                    ta = p1s.tile([128, nseg, 64], F32, tag=f"ra{toff}")
                    tb = p1s.tile([128, nseg, 64], F32, tag=f"rb{toff}")
                    nc.vector.tensor_mul(ta, qe, ce)
                    nc.gpsimd.tensor_mul(tb, qo, so)
                    tr = p1s.tile([128, nseg, 64], F32, tag=f"rr{toff}")
                    nc.vector.tensor_sub(tr, ta, tb)
                    nc.vector.tensor_mul(re, tr, rsb)
                    tcs = p1s.tile([128, nseg, 64], F32, tag=f"rc{toff}")
                    td = p1s.tile([128, nseg, 64], F32, tag=f"rd{toff}")
                    nc.gpsimd.tensor_mul(tcs, qe, se)
                    nc.vector.tensor_mul(td, qo, co)
                    to = p1s.tile([128, nseg, 64], F32, tag=f"ro{toff}")
                    nc.gpsimd.tensor_add(to, tcs, td)
                    nc.gpsimd.tensor_mul(ro, to, rsb)

                rope(0, 4, 0)   # q heads, tables 0..3
                rope(4, 1, 4)   # k, tables 4..7

                # transpose to [d, t] layout
                for sg in range(5):
                    tp = work.tile([128, 512], F32, tag="w")
                    nc.tensor.transpose(tp[:, 0:128], qr[:, sg * 128:(sg + 1) * 128],
                                        ident)
                    dst = qkT[:, sg, tt * 128:(tt + 1) * 128]
                    if sg < 3:
                        nc.scalar.activation(out=dst, in_=tp[:, 0:128],
                                             func=AF.Copy, scale=1.0, bias=0.0)
                    else:
                        nc.vector.tensor_copy(dst, tp[:, 0:128])

        def p2f(s):
            jmax = 4 * s + 3
            q0 = 512 * s
            for h in range(HQ):
                outT = outTp.tile([128, 512], F32, tag="outT")
                sums = sumsp.tile([128, 512], F32, tag="sums")
                for j in range(jmax + 1):
                    qlo = max(q0, j * 128)
                    w = q0 + 512 - qlo
                    poff = qlo - q0
                    sc = work.tile([128, 512], F32, tag="w")
                    nc.tensor.matmul(sc[:, 0:w], qkT[:, 4, j * 128:(j + 1) * 128],
                                     qkT[:, h, qlo:q0 + 512], start=True, stop=True)
                    if qlo == j * 128:
                        nc.vector.tensor_add(sc[:, 0:128], sc[:, 0:128], maskt_sb)
                    pT = p2.tile([128, 512], DT, tag="pT")
                    nc.scalar.activation(out=pT[:, 0:w], in_=sc[:, 0:w],
                                         func=AF.Exp, scale=SCALE)
                    nc.tensor.matmul(outT[:, poff:poff + w], vbuf[:, j, :],
                                     pT[:, 0:w], start=(j == 0), stop=(j == jmax),
                                     skip_group_check=True)
                    nc.tensor.matmul(sums[:, poff:poff + w], ones, pT[:, 0:w],
                                     start=(j == 0), stop=(j == jmax),
                                     skip_group_check=True)
                inv = p2.tile([128, 512], F32, tag="inv")
                nc.vector.reciprocal(out=inv, in_=sums)
                nc.vector.tensor_mul(attT[:, h, q0:q0 + 512], outT, inv)

        def p3f(s):
            for tt in range(4 * s, 4 * s + 4):
                for c4 in range(4):
                    y_ps = work.tile([128, 512], F32, tag="w")
                    for h in range(HQ):
                        nc.tensor.matmul(y_ps, attT[:, h, tt * 128:(tt + 1) * 128],
                                         wo_sb[:, h, c4 * 512:(c4 + 1) * 512],
                                         start=(h == 0), stop=(h == HQ - 1))
                    y_sb = p3.tile([128, 512], DT, tag="ysb")
                    if c4 % 2 == 0:
                        nc.vector.tensor_copy(y_sb, y_ps)
                    else:
                        nc.scalar.activation(out=y_sb, in_=y_ps, func=AF.Copy,
                                             scale=1.0, bias=0.0)
                    nc.sync.dma_start(
                        out=y[tt * 128:(tt + 1) * 128, c4 * 512:(c4 + 1) * 512],
                        in_=y_sb)

        p1(0)
        p1(1)
        for s in range(4):
            p2f(s)
            p3f(s)
            if s + 2 <= 3:
                p1(s + 2)

    nc.compile()
    return nc


def _prep_core_inputs(b, g, x, Wq, bq, Wk, bk, Wv, bv, Wo, bo, qn_w, kn_w,
                      freqs_cos, freqs_sin, mask, dt_name="float32r"):
    f32 = np.float32
    if dt_name == "bfloat16":
        import ml_dtypes
        dt_np = ml_dtypes.bfloat16
    else:
        dt_np = np.float32
    xb = np.ascontiguousarray(x[b], dtype=f32)
    # [tt, csub, cc, tcol]: xt[tt][p][cc*128+tc] = x[b][tt*128+tc][cc*128+p]
    xt = np.ascontiguousarray(
        xb.reshape(TB, 128, CB, 128).transpose(0, 3, 2, 1)
    ).reshape(TB, 128, CB * 128)
    wqkv = np.ascontiguousarray(np.concatenate([
        Wq[:, g * 512:(g + 1) * 512],
        Wk[:, g * 128:(g + 1) * 128],
        Wv[:, g * 128:(g + 1) * 128],
    ], axis=1).reshape(CB, 128, 768), dtype=f32)
    bqkv = np.concatenate([
        bq[g * 512:(g + 1) * 512], bk[g * 128:(g + 1) * 128],
        bv[g * 128:(g + 1) * 128],
    ]).reshape(1, 768).astype(f32)
    cos = freqs_cos.astype(f32)
    sin = freqs_sin.astype(f32)
    qe, qo = qn_w[0::2].astype(f32), qn_w[1::2].astype(f32)
    ke, ko = kn_w[0::2].astype(f32), kn_w[1::2].astype(f32)
    # tables: [ce, so, se, co] for q then for k; layout [TB, 128, 8*64]
    tabs = np.stack([cos * qe, sin * qo, sin * qe, cos * qo,
                     cos * ke, sin * ko, sin * ke, cos * ko], axis=1)  # [T, 8, 64]
    trig = np.ascontiguousarray(tabs.reshape(TB, 128, 8 * 64), dtype=f32)
    maskt = np.ascontiguousarray(mask[0, 0, :128, :128].T, dtype=f32)
    cst = np.stack([np.ones((128, 128), f32), np.eye(128, dtype=f32)])
    wo_t = np.ascontiguousarray(
        Wo[g * 512:(g + 1) * 512].reshape(HQ, 128, C), dtype=f32)
    out = {"xt": xt, "wqkv": wqkv, "bqkv": bqkv, "trig": trig,
           "maskt": maskt, "cst": cst, "wo": wo_t}
    if dt_np is not np.float32:
        for k in ("xt", "wqkv", "bqkv", "trig", "cst", "wo"):
            out[k] = out[k].astype(dt_np)
    return out


def kernel(x, Wq, bq, Wk, bk, Wv, bv, Wo, bo, qn_w, kn_w,
           freqs_cos, freqs_sin, mask, _trace=False, _trace_kwargs=None):
    from concourse.bass_utils import run_bass_kernel_spmd

    args = (np.asarray(x), np.asarray(Wq), np.asarray(bq), np.asarray(Wk),
            np.asarray(bk), np.asarray(Wv), np.asarray(bv), np.asarray(Wo),
            np.asarray(bo), np.asarray(qn_w), np.asarray(kn_w),
            np.asarray(freqs_cos), np.asarray(freqs_sin), np.asarray(mask))
    bo_np = args[8].astype(np.float32)

    dt_name = os.environ.get("BASS_ATTN_DT", "float32r")
    if dt_name not in _CACHE:
        _CACHE[dt_name] = _build(dt_name)
    nc = _CACHE[dt_name]

    in_maps = [_prep_core_inputs(cid // 4, cid % 4, *args, dt_name=dt_name)
               for cid in range(8)]
    res = run_bass_kernel_spmd(nc, in_maps, core_ids=list(range(8)),
                               trace=_trace, **(_trace_kwargs or {}))
    outs = [res.results[i]["y"] for i in range(8)]
    yfull = np.empty((B, T, C), dtype=np.float32)
    for b in range(B):
        yfull[b] = outs[4 * b] + outs[4 * b + 1] + outs[4 * b + 2] + outs[4 * b + 3]
        yfull[b] += bo_np[None, :]
    if _trace:
        kernel._last_result = res
    return yfull



# revision 9
# speedup vs baseline: 1.2694x; 1.1630x over previous
"""LLaMA causal self-attention (GQA) on 8 Trainium2 NeuronCores.

Sharding: 2-way data-parallel over batch x 4-way tensor-parallel over KV
groups. Core cid handles batch b=cid//4 and KV group g=cid%4 (q heads
4g..4g+3, kv head g). Each core computes a partial output y_partial =
att_heads @ Wo_rows; the host sums the 4 partials per batch and adds bo.

Per-core pipeline (all layouts chosen so matmul contraction is on the
partition dim and softmax needs no transposes):
  P1: QKV projection (x^T chunks stationary), bias via K=1 ones-row
      matmul, RMSNorm (ACT Square+accum, Newton-refined rsqrt), RoPE
      (elementwise, with qn_w/kn_w folded into the trig tables), then
      PE-transpose q/k to [d, t] layout.
  P2: attention computed transposed: scoresT[k, q] = kT_j^T @ qT chunks,
      additive causal mask on diagonal blocks, exp without max
      subtraction (RMS-normed scores are bounded, softmax is shift
      invariant), softmax denominators via an all-ones stationary matmul
      (sums land broadcast across all partitions), PV accumulated in
      PSUM over j, one normalize multiply per (head, half).
  P3: output projection from attT chunks, PSUM -> SBUF -> DRAM.
"""

import os
from contextlib import ExitStack

import numpy as np

B, T, C = 2, 2048, 2048
H, KV = 16, 4
D = 128
HQ = H // KV        # q heads per core = 4
TB = T // 128       # 16
CB = C // 128       # 16
EPS = 1e-5
SCALE = float(np.float32(1.0) / np.sqrt(np.float32(D)))

_CACHE = {}


def _build(dt_name):
    import concourse.bass as bass
    import concourse.bacc as bacc
    from concourse import mybir
    from concourse.tile import TileContext

    DT = getattr(mybir.dt, dt_name)
    F32 = mybir.dt.float32
    AF = mybir.ActivationFunctionType
    ALU = mybir.AluOpType

    nc = bacc.Bacc(None, target_bir_lowering=False)
    xt = nc.dram_tensor("xt", [TB, 128, CB * 128], DT, kind="ExternalInput")
    wqkv = nc.dram_tensor("wqkv", [CB, 128, 768], DT, kind="ExternalInput")
    bqkv = nc.dram_tensor("bqkv", [1, 768], DT, kind="ExternalInput")
    trig = nc.dram_tensor("trig", [TB, 128, 8 * 64], DT, kind="ExternalInput")
    maskt = nc.dram_tensor("maskt", [128, 128], F32, kind="ExternalInput")
    cst = nc.dram_tensor("cst", [2, 128, 128], DT, kind="ExternalInput")
    wo = nc.dram_tensor("wo", [HQ, 128, C], DT, kind="ExternalInput")
    y = nc.dram_tensor("y", [T, C], F32, kind="ExternalOutput")

    def bc4(apv, n):
        # broadcast a [128, 64] AP along a new middle (head) dim of size n
        return bass.AP(tensor=apv.tensor, offset=apv.offset,
                       ap=[list(apv.ap[0]), [0, n], [1, 64]])

    with TileContext(nc) as tc, ExitStack() as ctx:
        persist = ctx.enter_context(tc.tile_pool(name="persist", bufs=1))
        ones = persist.tile([128, 128], DT)
        ident = persist.tile([128, 128], DT)
        nc.sync.dma_start(out=ones, in_=cst[0])
        nc.sync.dma_start(out=ident, in_=cst[1])
        maskt_sb = persist.tile([128, 128], F32)
        nc.sync.dma_start(out=maskt_sb, in_=maskt[:, :])
        bq_sb = persist.tile([1, 768], DT)
        nc.sync.dma_start(out=bq_sb, in_=bqkv[:, :])
        # [d, seg, t]: segs 0..3 = q heads, seg 4 = k
        qkT = persist.tile([128, 5, T], DT)
        vbuf = persist.tile([128, TB, 128], DT)   # [t-in-block, j, d]
        attT = persist.tile([128, HQ, T], DT)     # [d, head, t]
        # ---------------- P1: QKV + RMSNorm + RoPE + transpose ----------
        with tc.tile_pool(name="p1", bufs=2) as p1, \
             tc.tile_pool(name="p1x", bufs=2) as p1x, \
             tc.tile_pool(name="p1s", bufs=3) as p1s, \
             tc.tile_pool(name="p1w", bufs=1) as p1w, \
             tc.tile_pool(name="p1ps", bufs=2, space="PSUM") as p1ps, \
             tc.tile_pool(name="p1tp", bufs=2, space="PSUM") as p1tp:
            wqkv_sb = p1w.tile([128, CB, 768], DT)
            for cc in range(CB):
                nc.sync.dma_start(out=wqkv_sb[:, cc, :], in_=wqkv[cc])
            for tt in range(TB):
                xtall = p1x.tile([128, CB, 128], DT, tag="xt")
                nc.sync.dma_start(out=xtall, in_=xt[tt])
                trig_sb = p1x.tile([128, 8, 64], DT, tag="trig")
                nc.sync.dma_start(out=trig_sb, in_=trig[tt])

                qkv_ps = p1ps.tile([128, 768], F32, tag="qkv")
                for cc in range(CB):
                    nc.tensor.matmul(qkv_ps[:, 0:512], xtall[:, cc, :],
                                     wqkv_sb[:, cc, 0:512],
                                     start=(cc == 0), stop=False)
                    nc.tensor.matmul(qkv_ps[:, 512:768], xtall[:, cc, :],
                                     wqkv_sb[:, cc, 512:768],
                                     start=(cc == 0), stop=False)
                nc.tensor.matmul(qkv_ps[:, 0:512], ones[0:1, :],
                                 bq_sb[0:1, 0:512], start=False, stop=True)
                nc.tensor.matmul(qkv_ps[:, 512:768], ones[0:1, :],
                                 bq_sb[0:1, 512:768], start=False, stop=True)

                # RMSNorm stats for 4 q heads + k
                ssq = p1s.tile([128, 8], F32, tag="ssq")
                for s in range(5):
                    sqs = p1s.tile([128, 128], F32, tag="sqs")
                    nc.scalar.activation(out=sqs, in_=qkv_ps[:, s * 128:(s + 1) * 128],
                                         func=AF.Square, accum_out=ssq[:, s:s + 1])
                x5 = p1s.tile([128, 8], F32, tag="x5")
                nc.scalar.activation(out=x5[:, 0:5], in_=ssq[:, 0:5],
                                     func=AF.Copy, scale=1.0 / D, bias=EPS)
                sq5 = p1s.tile([128, 8], F32, tag="sq5")
                nc.scalar.activation(out=sq5[:, 0:5], in_=x5[:, 0:5], func=AF.Sqrt)
                r0 = p1s.tile([128, 8], F32, tag="r0")
                nc.vector.reciprocal(out=r0[:, 0:5], in_=sq5[:, 0:5])
                # one Newton step: rstd = r0 * (1.5 - 0.5 * x * r0^2)
                t1 = p1s.tile([128, 8], F32, tag="t1")
                nc.vector.tensor_mul(t1[:, 0:5], r0[:, 0:5], r0[:, 0:5])
                t2 = p1s.tile([128, 8], F32, tag="t2")
                nc.vector.scalar_tensor_tensor(out=t2[:, 0:5], in0=t1[:, 0:5],
                                               scalar=-0.5, in1=x5[:, 0:5],
                                               op0=ALU.mult, op1=ALU.mult)
                t3 = p1s.tile([128, 8], F32, tag="t3")
                nc.vector.tensor_scalar_add(t3[:, 0:5], t2[:, 0:5], 1.5)
                rstd = p1s.tile([128, 8], F32, tag="rstd")
                nc.vector.tensor_mul(rstd[:, 0:5], r0[:, 0:5], t3[:, 0:5])

                # normalize q/k (x * rstd), copy v
                qn = p1.tile([128, 640], F32, tag="qn")
                for s in range(5):
                    nc.scalar.activation(out=qn[:, s * 128:(s + 1) * 128],
                                         in_=qkv_ps[:, s * 128:(s + 1) * 128],
                                         func=AF.Copy, scale=rstd[:, s:s + 1], bias=0.0)
                nc.scalar.activation(out=vbuf[:, tt, :], in_=qkv_ps[:, 640:768],
                                     func=AF.Copy, scale=1.0, bias=0.0)

                # RoPE (norm weights folded into trig tables host-side)
                qr = p1.tile([128, 640], DT, tag="qr")

                def rope(seg0, nseg, toff):
                    src = qn[:, seg0 * 128:(seg0 + nseg) * 128]
                    dst = qr[:, seg0 * 128:(seg0 + nseg) * 128]
                    sev = src.rearrange("p (h j t) -> p h t j", t=2, j=64)
                    dev = dst.rearrange("p (h j t) -> p h t j", t=2, j=64)
                    qe, qo = sev[:, :, 0, :], sev[:, :, 1, :]
                    re, ro = dev[:, :, 0, :], dev[:, :, 1, :]
                    ce = bc4(trig_sb[:, toff + 0, :], nseg)
                    so = bc4(trig_sb[:, toff + 1, :], nseg)
                    se = bc4(trig_sb[:, toff + 2, :], nseg)
                    co = bc4(trig_sb[:, toff + 3, :], nseg)
                    ta = p1s.tile([128, nseg, 64], F32, tag=f"ra{toff}")
                    tb = p1s.tile([128, nseg, 64], F32, tag=f"rb{toff}")
                    nc.vector.tensor_mul(ta, qe, ce)
                    nc.gpsimd.tensor_mul(tb, qo, so)
                    nc.vector.tensor_sub(re, ta, tb)
                    tc_ = p1s.tile([128, nseg, 64], F32, tag=f"rc{toff}")
                    td = p1s.tile([128, nseg, 64], F32, tag=f"rd{toff}")
                    nc.gpsimd.tensor_mul(tc_, qe, se)
                    nc.vector.tensor_mul(td, qo, co)
                    nc.gpsimd.tensor_add(ro, tc_, td)

                rope(0, 4, 0)   # q heads, tables 0..3
                rope(4, 1, 4)   # k, tables 4..7

                # transpose to [d, t]
                for s in range(5):
                    tp = p1tp.tile([128, 128], DT, tag="tp")
                    nc.tensor.transpose(tp, qr[:, s * 128:(s + 1) * 128], ident)
                    nc.scalar.activation(out=qkT[:, s, tt * 128:(tt + 1) * 128],
                                         in_=tp, func=AF.Copy, scale=1.0, bias=0.0)

        # ---------------- P2: attention (transposed scores) -------------
        with tc.tile_pool(name="p2", bufs=2) as p2, \
             tc.tile_pool(name="p2acc", bufs=1, space="PSUM") as p2acc, \
             tc.tile_pool(name="p2sc", bufs=2, space="PSUM") as p2sc:
            for h in range(HQ):
                for half in range(2):
                    q0 = half * 1024
                    q1 = q0 + 1024
                    jmax = 7 if half == 0 else 15
                    outT = p2acc.tile([128, 1024], F32, tag="outT")
                    sums = p2acc.tile([128, 1024], F32, tag="sums")
                    for j in range(jmax + 1):
                        qlo = max(q0, j * 128)
                        pT = p2.tile([128, 1024], DT, tag="pT")
                        kTj = qkT[:, 4, j * 128:(j + 1) * 128]
                        for ci in range(2):
                            begin = max(qlo, q0 + 512 * ci)
                            end = q0 + 512 * (ci + 1)
                            if begin >= end:
                                continue
                            w = end - begin
                            sc = p2sc.tile([128, 512], F32, tag="sc")
                            nc.tensor.matmul(sc[:, 0:w], kTj,
                                             qkT[:, h, begin:end],
                                             start=True, stop=True)
                            if begin == j * 128:
                                nc.vector.tensor_add(sc[:, 0:128],
                                                     sc[:, 0:128], maskt_sb)
                            poff = begin - qlo
                            nc.scalar.activation(out=pT[:, poff:poff + w],
                                                 in_=sc[:, 0:w], func=AF.Exp,
                                                 scale=SCALE)
                            nc.tensor.matmul(outT[:, begin - q0:begin - q0 + w],
                                             vbuf[:, j, :], pT[:, poff:poff + w],
                                             start=(j == 0), stop=(j == jmax),
                                             skip_group_check=True)
                            nc.tensor.matmul(sums[:, begin - q0:begin - q0 + w],
                                             ones, pT[:, poff:poff + w],
                                             start=(j == 0), stop=(j == jmax),
                                             skip_group_check=True)
                    inv = p2.tile([128, 1024], F32, tag="inv")
                    nc.vector.reciprocal(out=inv, in_=sums)
                    nc.vector.tensor_mul(attT[:, h, q0:q1], outT, inv)

        # ---------------- P3: output projection --------------------------
        with tc.tile_pool(name="p3", bufs=2) as p3, \
             tc.tile_pool(name="p3w", bufs=1) as p3w, \
             tc.tile_pool(name="p3ps", bufs=2, space="PSUM") as p3ps:
            wo_sb = p3w.tile([128, HQ, C], DT)
            for h in range(HQ):
                nc.sync.dma_start(out=wo_sb[:, h, :], in_=wo[h])
            for tt in range(TB):
                y_ps = p3ps.tile([128, 2048], F32, tag="y")
                for h in range(HQ):
                    for c4 in range(4):
                        nc.tensor.matmul(y_ps[:, c4 * 512:(c4 + 1) * 512],
                                         attT[:, h, tt * 128:(tt + 1) * 128],
                                         wo_sb[:, h, c4 * 512:(c4 + 1) * 512],
                                         start=(h == 0), stop=(h == HQ - 1))
                y_sb = p3.tile([128, 2048], F32, tag="ysb")
                nc.scalar.activation(out=y_sb[:, 0:1024], in_=y_ps[:, 0:1024],
                                     func=AF.Copy, scale=1.0, bias=0.0)
                nc.vector.tensor_copy(y_sb[:, 1024:2048], y_ps[:, 1024:2048])
                nc.sync.dma_start(out=y[tt * 128:(tt + 1) * 128, :], in_=y_sb)

    nc.compile()
    return nc


def _build_fused(dt_name):
    """Fused software pipeline: P1 (QKV+RMSNorm+RoPE+transpose), P2
    (attention) and P3 (output proj) interleaved over 512-row q-stripes
    so the tensor engine stays dense and warm.  Emission order:
    P1(0) P1(1) [P2(s) P3(s) P1(s+2)] for s=0..3.

    PSUM budget (8 banks): qkv accumulator [128,768] x1 = 2, shared
    work pool [128,512] x4 = 4 (scores / transposes / y chunks),
    outT x1 = 1, sums x1 = 1.
    """
    import concourse.bass as bass
    import concourse.bacc as bacc
    from concourse import mybir
    from concourse.tile import TileContext

    DT = getattr(mybir.dt, dt_name)
    F32 = mybir.dt.float32
    AF = mybir.ActivationFunctionType

    nc = bacc.Bacc(None, target_bir_lowering=False)
    xt = nc.dram_tensor("xt", [TB, 128, CB * 128], DT, kind="ExternalInput")
    wqkv = nc.dram_tensor("wqkv", [CB, 128, 768], DT, kind="ExternalInput")
    bqkv = nc.dram_tensor("bqkv", [1, 768], DT, kind="ExternalInput")
    trig = nc.dram_tensor("trig", [TB, 128, 8 * 64], F32, kind="ExternalInput")
    maskt = nc.dram_tensor("maskt", [128, 128], F32, kind="ExternalInput")
    cst = nc.dram_tensor("cst", [2, 128, 128], DT, kind="ExternalInput")
    wo = nc.dram_tensor("wo", [HQ, 128, C], DT, kind="ExternalInput")
    y = nc.dram_tensor("y", [T, C], DT, kind="ExternalOutput")

    def bc(apv, n, w=64):
        # broadcast a [128, w] AP along a new middle dim of size n
        return bass.AP(tensor=apv.tensor, offset=apv.offset,
                       ap=[list(apv.ap[0]), [0, n], [1, w]])

    with TileContext(nc) as tc, ExitStack() as ctx:
        persist = ctx.enter_context(tc.tile_pool(name="persist", bufs=1))
        p1x = ctx.enter_context(tc.tile_pool(name="p1x", bufs=3))
        p1s = ctx.enter_context(tc.tile_pool(name="p1s", bufs=3))
        p2 = ctx.enter_context(tc.tile_pool(name="p2", bufs=3))
        p3 = ctx.enter_context(tc.tile_pool(name="p3", bufs=3))
        p1ps = ctx.enter_context(tc.tile_pool(name="p1ps", bufs=1, space="PSUM"))
        work = ctx.enter_context(tc.tile_pool(name="work", bufs=4, space="PSUM"))
        outTp = ctx.enter_context(tc.tile_pool(name="outTp", bufs=1, space="PSUM"))
        sumsp = ctx.enter_context(tc.tile_pool(name="sumsp", bufs=1, space="PSUM"))

        ones = persist.tile([128, 128], DT)
        ident = persist.tile([128, 128], DT)
        nc.sync.dma_start(out=ones, in_=cst[0])
        nc.sync.dma_start(out=ident, in_=cst[1])
        maskt_sb = persist.tile([128, 128], F32)
        nc.sync.dma_start(out=maskt_sb, in_=maskt[:, :])
        bq_sb = persist.tile([1, 768], DT)
        nc.sync.dma_start(out=bq_sb, in_=bqkv[:, :])
        # [d, seg, t]: segs 0..3 = q heads, seg 4 = k
        qkT = persist.tile([128, 5, T], DT)
        vbuf = persist.tile([128, TB, 128], DT)   # [t-in-block, j, d]
        attT = persist.tile([128, HQ, T], DT)     # [d, head, t]
        wqkv_sb = persist.tile([128, CB, 768], DT)
        for cc in range(CB):
            nc.sync.dma_start(out=wqkv_sb[:, cc, :], in_=wqkv[cc])
        wo_sb = persist.tile([128, HQ, C], DT)
        for h in range(HQ):
            nc.sync.dma_start(out=wo_sb[:, h, :], in_=wo[h])

        def p1(s):
            for tt in range(4 * s, 4 * s + 4):
                xtall = p1x.tile([128, CB, 128], DT, tag="xt")
                nc.sync.dma_start(out=xtall, in_=xt[tt])
                trig_sb = p1x.tile([128, 8, 64], F32, tag="trig")
                nc.sync.dma_start(out=trig_sb, in_=trig[tt])

                qkv_ps = p1ps.tile([128, 768], F32, tag="qkv")
                for cc in range(CB):
                    nc.tensor.matmul(qkv_ps[:, 0:512], xtall[:, cc, :],
                                     wqkv_sb[:, cc, 0:512],
                                     start=(cc == 0), stop=False)
                    nc.tensor.matmul(qkv_ps[:, 512:768], xtall[:, cc, :],
                                     wqkv_sb[:, cc, 512:768],
                                     start=(cc == 0), stop=False)
                nc.tensor.matmul(qkv_ps[:, 0:512], ones[0:1, :],
                                 bq_sb[0:1, 0:512], start=False, stop=True)
                nc.tensor.matmul(qkv_ps[:, 512:768], ones[0:1, :],
                                 bq_sb[0:1, 512:768], start=False, stop=True)

                # evacuate PSUM fast: q/k raw to SBUF f32, v to vbuf bf16
                qsb = p1s.tile([128, 640], F32, tag="qsb")
                nc.scalar.activation(out=qsb[:, 0:384], in_=qkv_ps[:, 0:384],
                                     func=AF.Copy, scale=1.0, bias=0.0)
                nc.vector.tensor_copy(qsb[:, 384:640], qkv_ps[:, 384:640])
                nc.vector.tensor_copy(vbuf[:, tt, :], qkv_ps[:, 640:768])

                # RMSNorm stats (5 segs: 4 q heads + k)
                ssq = p1s.tile([128, 8], F32, tag="ssq")
                for sg in range(5):
                    sqs = p1s.tile([128, 128], F32, tag="sqs")
                    nc.scalar.activation(out=sqs, in_=qsb[:, sg * 128:(sg + 1) * 128],
                                         func=AF.Square, accum_out=ssq[:, sg:sg + 1])
                x5 = p1s.tile([128, 8], F32, tag="x5")
                nc.scalar.activation(out=x5[:, 0:5], in_=ssq[:, 0:5],
                                     func=AF.Copy, scale=1.0 / D, bias=EPS)
                sq5 = p1s.tile([128, 8], F32, tag="sq5")
                nc.scalar.activation(out=sq5[:, 0:5], in_=x5[:, 0:5], func=AF.Sqrt)
                rstd = p1s.tile([128, 8], F32, tag="rstd")
                nc.vector.reciprocal(out=rstd[:, 0:5], in_=sq5[:, 0:5])

                # RoPE with rstd folded in post-combine; qn_w/kn_w are in trig
                qr = p1s.tile([128, 640], DT, tag="qr")

                def rope(seg0, nseg, toff):
                    src = qsb[:, seg0 * 128:(seg0 + nseg) * 128]
                    dst = qr[:, seg0 * 128:(seg0 + nseg) * 128]
                    sev = src.rearrange("p (h j t) -> p h t j", t=2, j=64)
                    dev = dst.rearrange("p (h j t) -> p h t j", t=2, j=64)
                    qe, qo = sev[:, :, 0, :], sev[:, :, 1, :]
                    re, ro = dev[:, :, 0, :], dev[:, :, 1, :]
                    ce = bc(trig_sb[:, toff + 0, :], nseg)
                    so = bc(trig_sb[:, toff + 1, :], nseg)
                    se = bc(trig_sb[:, toff + 2, :], nseg)
                    co = bc(trig_sb[:, toff + 3, :], nseg)
                    rsb = bass.AP(tensor=rstd.tensor, offset=rstd.offset + seg0,
                                  ap=[list(rstd.ap[0]), [1, nseg], [0, 64]])
                    ta = p1s.tile([128, nseg, 64], F32, tag=f"ra{toff}")
                    tb = p1s.tile([128, nseg, 64], F32, tag=f"rb{toff}")
                    nc.vector.tensor_mul(ta, qe, ce)
                    nc.gpsimd.tensor_mul(tb, qo, so)
                    tr = p1s.tile([128, nseg, 64], F32, tag=f"rr{toff}")
                    nc.vector.tensor_sub(tr, ta, tb)
                    nc.vector.tensor_mul(re, tr, rsb)
                    tcs = p1s.tile([128, nseg, 64], F32, tag=f"rc{toff}")
                    td = p1s.tile([128, nseg, 64], F32, tag=f"rd{toff}")
                    nc.gpsimd.tensor_mul(tcs, qe, se)
                    nc.vector.tensor_mul(td, qo, co)
                    to = p1s.tile([128, nseg, 64], F32, tag=f"ro{toff}")
                    nc.gpsimd.tensor_add(to, tcs, td)
                    nc.gpsimd.tensor_mul(ro, to, rsb)

                rope(0, 4, 0)   # q heads, tables 0..3
                rope(4, 1, 4)   # k, tables 4..7

                # transpose to [d, t] layout
                for sg in range(5):
                    tp = work.tile([128, 512], DT, tag="w")
                    nc.tensor.transpose(tp[:, 0:128], qr[:, sg * 128:(sg + 1) * 128],
                                        ident)
                    dst = qkT[:, sg, tt * 128:(tt + 1) * 128]
                    if sg < 3:
                        nc.scalar.activation(out=dst, in_=tp[:, 0:128],
                                             func=AF.Copy, scale=1.0, bias=0.0)
                    else:
                        nc.vector.tensor_copy(dst, tp[:, 0:128])

        def p2f(s):
            jmax = 4 * s + 3
            q0 = 512 * s
            for h in range(HQ):
                outT = outTp.tile([128, 512], F32, tag="outT")
                sums = sumsp.tile([128, 512], F32, tag="sums")
                for j in range(jmax + 1):
                    qlo = max(q0, j * 128)
                    w = q0 + 512 - qlo
                    poff = qlo - q0
                    sc = work.tile([128, 512], F32, tag="w")
                    nc.tensor.matmul(sc[:, 0:w], qkT[:, 4, j * 128:(j + 1) * 128],
                                     qkT[:, h, qlo:q0 + 512], start=True, stop=True)
                    if qlo == j * 128:
                        nc.vector.tensor_add(sc[:, 0:128], sc[:, 0:128], maskt_sb)
                    pT = p2.tile([128, 512], DT, tag="pT")
                    nc.scalar.activation(out=pT[:, 0:w], in_=sc[:, 0:w],
                                         func=AF.Exp, scale=SCALE)
                    nc.tensor.matmul(outT[:, poff:poff + w], vbuf[:, j, :],
                                     pT[:, 0:w], start=(j == 0), stop=(j == jmax),
                                     skip_group_check=True)
                    nc.tensor.matmul(sums[:, poff:poff + w], ones, pT[:, 0:w],
                                     start=(j == 0), stop=(j == jmax),
                                     skip_group_check=True)
                inv = p2.tile([128, 512], F32, tag="inv")
                nc.vector.reciprocal(out=inv, in_=sums)
                nc.vector.tensor_mul(attT[:, h, q0:q0 + 512], outT, inv)

        def p3f(s):
            for tt in range(4 * s, 4 * s + 4):
                for c4 in range(4):
                    y_ps = work.tile([128, 512], F32, tag="w")
                    for h in range(HQ):
                        nc.tensor.matmul(y_ps, attT[:, h, tt * 128:(tt + 1) * 128],
                                         wo_sb[:, h, c4 * 512:(c4 + 1) * 512],
                                         start=(h == 0), stop=(h == HQ - 1))
                    y_sb = p3.tile([128, 512], DT, tag="ysb")
                    if c4 % 2 == 0:
                        nc.vector.tensor_copy(y_sb, y_ps)
                    else:
                        nc.scalar.activation(out=y_sb, in_=y_ps, func=AF.Copy,
                                             scale=1.0, bias=0.0)
                    nc.sync.dma_start(
                        out=y[tt * 128:(tt + 1) * 128, c4 * 512:(c4 + 1) * 512],
                        in_=y_sb)

        p1(0)
        p1(1)
        for s in range(4):
            p2f(s)
            p3f(s)
            if s + 2 <= 3:
                p1(s + 2)

    nc.compile()
    return nc


def _prep_core_inputs(b, g, x, Wq, bq, Wk, bk, Wv, bv, Wo, bo, qn_w, kn_w,
                      freqs_cos, freqs_sin, mask, dt_name="float32r",
                      impl="base"):
    f32 = np.float32
    if dt_name == "bfloat16":
        import ml_dtypes
        dt_np = ml_dtypes.bfloat16
    else:
        dt_np = np.float32
    xb = np.ascontiguousarray(x[b], dtype=f32)
    # [tt, csub, cc, tcol]: xt[tt][p][cc*128+tc] = x[b][tt*128+tc][cc*128+p]
    xt = np.ascontiguousarray(
        xb.reshape(TB, 128, CB, 128).transpose(0, 3, 2, 1)
    ).reshape(TB, 128, CB * 128)
    wqkv = np.ascontiguousarray(np.concatenate([
        Wq[:, g * 512:(g + 1) * 512],
        Wk[:, g * 128:(g + 1) * 128],
        Wv[:, g * 128:(g + 1) * 128],
    ], axis=1).reshape(CB, 128, 768), dtype=f32)
    bqkv = np.concatenate([
        bq[g * 512:(g + 1) * 512], bk[g * 128:(g + 1) * 128],
        bv[g * 128:(g + 1) * 128],
    ]).reshape(1, 768).astype(f32)
    cos = freqs_cos.astype(f32)
    sin = freqs_sin.astype(f32)
    qe, qo = qn_w[0::2].astype(f32), qn_w[1::2].astype(f32)
    ke, ko = kn_w[0::2].astype(f32), kn_w[1::2].astype(f32)
    # tables: [ce, so, se, co] for q then for k; layout [TB, 128, 8*64]
    tabs = np.stack([cos * qe, sin * qo, sin * qe, cos * qo,
                     cos * ke, sin * ko, sin * ke, cos * ko], axis=1)  # [T, 8, 64]
    trig = np.ascontiguousarray(tabs.reshape(TB, 128, 8 * 64), dtype=f32)
    maskt = np.ascontiguousarray(mask[0, 0, :128, :128].T, dtype=f32)
    cst = np.stack([np.ones((128, 128), f32), np.eye(128, dtype=f32)])
    wo_t = np.ascontiguousarray(
        Wo[g * 512:(g + 1) * 512].reshape(HQ, 128, C), dtype=f32)
    out = {"xt": xt, "wqkv": wqkv, "bqkv": bqkv, "trig": trig,
           "maskt": maskt, "cst": cst, "wo": wo_t}
    if dt_np is not np.float32:
        keys = ("xt", "wqkv", "bqkv", "cst", "wo") if impl == "fused" else \
               ("xt", "wqkv", "bqkv", "trig", "cst", "wo")
        for k in keys:
            out[k] = out[k].astype(dt_np)
    return out


def kernel(x, Wq, bq, Wk, bk, Wv, bv, Wo, bo, qn_w, kn_w,
           freqs_cos, freqs_sin, mask, _trace=False, _trace_kwargs=None):
    from concourse.bass_utils import run_bass_kernel_spmd

    args = (np.asarray(x), np.asarray(Wq), np.asarray(bq), np.asarray(Wk),
            np.asarray(bk), np.asarray(Wv), np.asarray(bv), np.asarray(Wo),
            np.asarray(bo), np.asarray(qn_w), np.asarray(kn_w),
            np.asarray(freqs_cos), np.asarray(freqs_sin), np.asarray(mask))
    bo_np = args[8].astype(np.float32)

    impl = os.environ.get("BASS_ATTN_IMPL", "base")
    dt_name = os.environ.get("BASS_ATTN_DT", "float32r")
    key = (impl, dt_name)
    if key not in _CACHE:
        _CACHE[key] = _build_fused(dt_name) if impl == "fused" else _build(dt_name)
    nc = _CACHE[key]

    in_maps = [_prep_core_inputs(cid // 4, cid % 4, *args, dt_name=dt_name,
                                 impl=impl)
               for cid in range(8)]
    res = run_bass_kernel_spmd(nc, in_maps, core_ids=list(range(8)),
                               trace=_trace, **(_trace_kwargs or {}))
    outs = [np.asarray(res.results[i]["y"], dtype=np.float32) for i in range(8)]
    yfull = np.empty((B, T, C), dtype=np.float32)
    for b in range(B):
        yfull[b] = outs[4 * b] + outs[4 * b + 1] + outs[4 * b + 2] + outs[4 * b + 3]
        yfull[b] += bo_np[None, :]
    if _trace:
        kernel._last_result = res
    return yfull



# revision 15
# speedup vs baseline: 1.5028x; 1.1839x over previous
"""LLaMA causal self-attention (GQA) on 8 Trainium2 NeuronCores.

Sharding: 2-way data-parallel over batch x 4-way tensor-parallel over KV
groups. Core cid handles batch b=cid//4 and KV group g=cid%4 (q heads
4g..4g+3, kv head g). Each core computes a partial output y_partial =
att_heads @ Wo_rows; the host sums the 4 partials per batch and adds bo.

Per-core pipeline (all layouts chosen so matmul contraction is on the
partition dim and softmax needs no transposes):
  P1: QKV projection (x^T chunks stationary), bias via K=1 ones-row
      matmul, RMSNorm (ACT Square+accum, Newton-refined rsqrt), RoPE
      (elementwise, with qn_w/kn_w folded into the trig tables), then
      PE-transpose q/k to [d, t] layout.
  P2: attention computed transposed: scoresT[k, q] = kT_j^T @ qT chunks,
      additive causal mask on diagonal blocks, exp without max
      subtraction (RMS-normed scores are bounded, softmax is shift
      invariant), softmax denominators via an all-ones stationary matmul
      (sums land broadcast across all partitions), PV accumulated in
      PSUM over j, one normalize multiply per (head, half).
  P3: output projection from attT chunks, PSUM -> SBUF -> DRAM.
"""

import os
from contextlib import ExitStack

import numpy as np

B, T, C = 2, 2048, 2048
H, KV = 16, 4
D = 128
HQ = H // KV        # q heads per core = 4
TB = T // 128       # 16
CB = C // 128       # 16
EPS = 1e-5
SCALE = float(np.float32(1.0) / np.sqrt(np.float32(D)))

_CACHE = {}


def _build(dt_name):
    import concourse.bass as bass
    import concourse.bacc as bacc
    from concourse import mybir
    from concourse.tile import TileContext

    DT = getattr(mybir.dt, dt_name)
    F32 = mybir.dt.float32
    AF = mybir.ActivationFunctionType
    ALU = mybir.AluOpType

    nc = bacc.Bacc(None, target_bir_lowering=False)
    xt = nc.dram_tensor("xt", [TB, 128, CB * 128], DT, kind="ExternalInput")
    wqkv = nc.dram_tensor("wqkv", [CB, 128, 768], DT, kind="ExternalInput")
    bqkv = nc.dram_tensor("bqkv", [1, 768], DT, kind="ExternalInput")
    trig = nc.dram_tensor("trig", [TB, 128, 8 * 64], DT, kind="ExternalInput")
    maskt = nc.dram_tensor("maskt", [128, 128], F32, kind="ExternalInput")
    cst = nc.dram_tensor("cst", [2, 128, 128], DT, kind="ExternalInput")
    wo = nc.dram_tensor("wo", [HQ, 128, C], DT, kind="ExternalInput")
    y = nc.dram_tensor("y", [T, C], F32, kind="ExternalOutput")

    def bc4(apv, n):
        # broadcast a [128, 64] AP along a new middle (head) dim of size n
        return bass.AP(tensor=apv.tensor, offset=apv.offset,
                       ap=[list(apv.ap[0]), [0, n], [1, 64]])

    with TileContext(nc) as tc, ExitStack() as ctx:
        persist = ctx.enter_context(tc.tile_pool(name="persist", bufs=1))
        ones = persist.tile([128, 128], DT)
        ident = persist.tile([128, 128], DT)
        nc.sync.dma_start(out=ones, in_=cst[0])
        nc.sync.dma_start(out=ident, in_=cst[1])
        maskt_sb = persist.tile([128, 128], F32)
        nc.sync.dma_start(out=maskt_sb, in_=maskt[:, :])
        bq_sb = persist.tile([1, 768], DT)
        nc.sync.dma_start(out=bq_sb, in_=bqkv[:, :])
        # [d, seg, t]: segs 0..3 = q heads, seg 4 = k
        qkT = persist.tile([128, 5, T], DT)
        vbuf = persist.tile([128, TB, 128], DT)   # [t-in-block, j, d]
        attT = persist.tile([128, HQ, T], DT)     # [d, head, t]
        # ---------------- P1: QKV + RMSNorm + RoPE + transpose ----------
        with tc.tile_pool(name="p1", bufs=2) as p1, \
             tc.tile_pool(name="p1x", bufs=2) as p1x, \
             tc.tile_pool(name="p1s", bufs=3) as p1s, \
             tc.tile_pool(name="p1w", bufs=1) as p1w, \
             tc.tile_pool(name="p1ps", bufs=2, space="PSUM") as p1ps, \
             tc.tile_pool(name="p1tp", bufs=2, space="PSUM") as p1tp:
            wqkv_sb = p1w.tile([128, CB, 768], DT)
            for cc in range(CB):
                nc.sync.dma_start(out=wqkv_sb[:, cc, :], in_=wqkv[cc])
            for tt in range(TB):
                xtall = p1x.tile([128, CB, 128], DT, tag="xt")
                nc.sync.dma_start(out=xtall, in_=xt[tt])
                trig_sb = p1x.tile([128, 8, 64], DT, tag="trig")
                nc.sync.dma_start(out=trig_sb, in_=trig[tt])

                qkv_ps = p1ps.tile([128, 768], F32, tag="qkv")
                for cc in range(CB):
                    nc.tensor.matmul(qkv_ps[:, 0:512], xtall[:, cc, :],
                                     wqkv_sb[:, cc, 0:512],
                                     start=(cc == 0), stop=False)
                    nc.tensor.matmul(qkv_ps[:, 512:768], xtall[:, cc, :],
                                     wqkv_sb[:, cc, 512:768],
                                     start=(cc == 0), stop=False)
                nc.tensor.matmul(qkv_ps[:, 0:512], ones[0:1, :],
                                 bq_sb[0:1, 0:512], start=False, stop=True)
                nc.tensor.matmul(qkv_ps[:, 512:768], ones[0:1, :],
                                 bq_sb[0:1, 512:768], start=False, stop=True)

                # RMSNorm stats for 4 q heads + k
                ssq = p1s.tile([128, 8], F32, tag="ssq")
                for s in range(5):
                    sqs = p1s.tile([128, 128], F32, tag="sqs")
                    nc.scalar.activation(out=sqs, in_=qkv_ps[:, s * 128:(s + 1) * 128],
                                         func=AF.Square, accum_out=ssq[:, s:s + 1])
                x5 = p1s.tile([128, 8], F32, tag="x5")
                nc.scalar.activation(out=x5[:, 0:5], in_=ssq[:, 0:5],
                                     func=AF.Copy, scale=1.0 / D, bias=EPS)
                sq5 = p1s.tile([128, 8], F32, tag="sq5")
                nc.scalar.activation(out=sq5[:, 0:5], in_=x5[:, 0:5], func=AF.Sqrt)
                r0 = p1s.tile([128, 8], F32, tag="r0")
                nc.vector.reciprocal(out=r0[:, 0:5], in_=sq5[:, 0:5])
                # one Newton step: rstd = r0 * (1.5 - 0.5 * x * r0^2)
                t1 = p1s.tile([128, 8], F32, tag="t1")
                nc.vector.tensor_mul(t1[:, 0:5], r0[:, 0:5], r0[:, 0:5])
                t2 = p1s.tile([128, 8], F32, tag="t2")
                nc.vector.scalar_tensor_tensor(out=t2[:, 0:5], in0=t1[:, 0:5],
                                               scalar=-0.5, in1=x5[:, 0:5],
                                               op0=ALU.mult, op1=ALU.mult)
                t3 = p1s.tile([128, 8], F32, tag="t3")
                nc.vector.tensor_scalar_add(t3[:, 0:5], t2[:, 0:5], 1.5)
                rstd = p1s.tile([128, 8], F32, tag="rstd")
                nc.vector.tensor_mul(rstd[:, 0:5], r0[:, 0:5], t3[:, 0:5])

                # normalize q/k (x * rstd), copy v
                qn = p1.tile([128, 640], F32, tag="qn")
                for s in range(5):
                    nc.scalar.activation(out=qn[:, s * 128:(s + 1) * 128],
                                         in_=qkv_ps[:, s * 128:(s + 1) * 128],
                                         func=AF.Copy, scale=rstd[:, s:s + 1], bias=0.0)
                nc.scalar.activation(out=vbuf[:, tt, :], in_=qkv_ps[:, 640:768],
                                     func=AF.Copy, scale=1.0, bias=0.0)

                # RoPE (norm weights folded into trig tables host-side)
                qr = p1.tile([128, 640], DT, tag="qr")

                def rope(seg0, nseg, toff):
                    src = qn[:, seg0 * 128:(seg0 + nseg) * 128]
                    dst = qr[:, seg0 * 128:(seg0 + nseg) * 128]
                    sev = src.rearrange("p (h j t) -> p h t j", t=2, j=64)
                    dev = dst.rearrange("p (h j t) -> p h t j", t=2, j=64)
                    qe, qo = sev[:, :, 0, :], sev[:, :, 1, :]
                    re, ro = dev[:, :, 0, :], dev[:, :, 1, :]
                    ce = bc4(trig_sb[:, toff + 0, :], nseg)
                    so = bc4(trig_sb[:, toff + 1, :], nseg)
                    se = bc4(trig_sb[:, toff + 2, :], nseg)
                    co = bc4(trig_sb[:, toff + 3, :], nseg)
                    ta = p1s.tile([128, nseg, 64], F32, tag=f"ra{toff}")
                    tb = p1s.tile([128, nseg, 64], F32, tag=f"rb{toff}")
                    nc.vector.tensor_mul(ta, qe, ce)
                    nc.gpsimd.tensor_mul(tb, qo, so)
                    nc.vector.tensor_sub(re, ta, tb)
                    tc_ = p1s.tile([128, nseg, 64], F32, tag=f"rc{toff}")
                    td = p1s.tile([128, nseg, 64], F32, tag=f"rd{toff}")
                    nc.gpsimd.tensor_mul(tc_, qe, se)
                    nc.vector.tensor_mul(td, qo, co)
                    nc.gpsimd.tensor_add(ro, tc_, td)

                rope(0, 4, 0)   # q heads, tables 0..3
                rope(4, 1, 4)   # k, tables 4..7

                # transpose to [d, t]
                for s in range(5):
                    tp = p1tp.tile([128, 128], DT, tag="tp")
                    nc.tensor.transpose(tp, qr[:, s * 128:(s + 1) * 128], ident)
                    nc.scalar.activation(out=qkT[:, s, tt * 128:(tt + 1) * 128],
                                         in_=tp, func=AF.Copy, scale=1.0, bias=0.0)

        # ---------------- P2: attention (transposed scores) -------------
        with tc.tile_pool(name="p2", bufs=2) as p2, \
             tc.tile_pool(name="p2acc", bufs=1, space="PSUM") as p2acc, \
             tc.tile_pool(name="p2sc", bufs=2, space="PSUM") as p2sc:
            for h in range(HQ):
                for half in range(2):
                    q0 = half * 1024
                    q1 = q0 + 1024
                    jmax = 7 if half == 0 else 15
                    outT = p2acc.tile([128, 1024], F32, tag="outT")
                    sums = p2acc.tile([128, 1024], F32, tag="sums")
                    for j in range(jmax + 1):
                        qlo = max(q0, j * 128)
                        pT = p2.tile([128, 1024], DT, tag="pT")
                        kTj = qkT[:, 4, j * 128:(j + 1) * 128]
                        for ci in range(2):
                            begin = max(qlo, q0 + 512 * ci)
                            end = q0 + 512 * (ci + 1)
                            if begin >= end:
                                continue
                            w = end - begin
                            sc = p2sc.tile([128, 512], F32, tag="sc")
                            nc.tensor.matmul(sc[:, 0:w], kTj,
                                             qkT[:, h, begin:end],
                                             start=True, stop=True)
                            if begin == j * 128:
                                nc.vector.tensor_add(sc[:, 0:128],
                                                     sc[:, 0:128], maskt_sb)
                            poff = begin - qlo
                            nc.scalar.activation(out=pT[:, poff:poff + w],
                                                 in_=sc[:, 0:w], func=AF.Exp,
                                                 scale=SCALE)
                            nc.tensor.matmul(outT[:, begin - q0:begin - q0 + w],
                                             vbuf[:, j, :], pT[:, poff:poff + w],
                                             start=(j == 0), stop=(j == jmax),
                                             skip_group_check=True)
                            nc.tensor.matmul(sums[:, begin - q0:begin - q0 + w],
                                             ones, pT[:, poff:poff + w],
                                             start=(j == 0), stop=(j == jmax),
                                             skip_group_check=True)
                    inv = p2.tile([128, 1024], F32, tag="inv")
                    nc.vector.reciprocal(out=inv, in_=sums)
                    nc.vector.tensor_mul(attT[:, h, q0:q1], outT, inv)

        # ---------------- P3: output projection --------------------------
        with tc.tile_pool(name="p3", bufs=2) as p3, \
             tc.tile_pool(name="p3w", bufs=1) as p3w, \
             tc.tile_pool(name="p3ps", bufs=2, space="PSUM") as p3ps:
            wo_sb = p3w.tile([128, HQ, C], DT)
            for h in range(HQ):
                nc.sync.dma_start(out=wo_sb[:, h, :], in_=wo[h])
            for tt in range(TB):
                y_ps = p3ps.tile([128, 2048], F32, tag="y")
                for h in range(HQ):
                    for c4 in range(4):
                        nc.tensor.matmul(y_ps[:, c4 * 512:(c4 + 1) * 512],
                                         attT[:, h, tt * 128:(tt + 1) * 128],
                                         wo_sb[:, h, c4 * 512:(c4 + 1) * 512],
                                         start=(h == 0), stop=(h == HQ - 1))
                y_sb = p3.tile([128, 2048], F32, tag="ysb")
                nc.scalar.activation(out=y_sb[:, 0:1024], in_=y_ps[:, 0:1024],
                                     func=AF.Copy, scale=1.0, bias=0.0)
                nc.vector.tensor_copy(y_sb[:, 1024:2048], y_ps[:, 1024:2048])
                nc.sync.dma_start(out=y[tt * 128:(tt + 1) * 128, :], in_=y_sb)

    nc.compile()
    return nc


def _build_fused(dt_name):
    """Fused software pipeline: P1 (QKV+RMSNorm+RoPE+transpose), P2
    (attention) and P3 (output proj) interleaved over 512-row q-stripes
    so the tensor engine stays dense and warm.  Emission order:
    P1(0) P1(1) [P2(s) P3(s) P1(s+2)] for s=0..3.

    PSUM budget (8 banks): qkv accumulator [128,768] x1 = 2, shared
    work pool [128,512] x4 = 4 (scores / transposes / y chunks),
    outT x1 = 1, sums x1 = 1.
    """
    import concourse.bass as bass
    import concourse.bacc as bacc
    from concourse import mybir
    from concourse.tile import TileContext

    DT = getattr(mybir.dt, dt_name)
    F32 = mybir.dt.float32
    AF = mybir.ActivationFunctionType

    nc = bacc.Bacc(None, target_bir_lowering=False)
    xt = nc.dram_tensor("xt", [TB, 128, CB * 128], DT, kind="ExternalInput")
    wqkv = nc.dram_tensor("wqkv", [CB, 128, 768], DT, kind="ExternalInput")
    bqkv = nc.dram_tensor("bqkv", [1, 768], DT, kind="ExternalInput")
    trig = nc.dram_tensor("trig", [TB, 128, 8 * 64], DT, kind="ExternalInput")
    maskt = nc.dram_tensor("maskt", [128, 128], DT, kind="ExternalInput")
    cst = nc.dram_tensor("cst", [2, 128, 128], DT, kind="ExternalInput")
    wo = nc.dram_tensor("wo", [HQ, 128, C], DT, kind="ExternalInput")
    y = nc.dram_tensor("y", [T, C], DT, kind="ExternalOutput")

    def bc(apv, n, w=64):
        # broadcast a [128, w] AP along a new middle dim of size n
        return bass.AP(tensor=apv.tensor, offset=apv.offset,
                       ap=[list(apv.ap[0]), [0, n], [1, w]])

    with TileContext(nc) as tc, ExitStack() as ctx:
        persist = ctx.enter_context(tc.tile_pool(name="persist", bufs=1))
        p1x = ctx.enter_context(tc.tile_pool(name="p1x", bufs=3))
        p1s = ctx.enter_context(tc.tile_pool(name="p1s", bufs=3))
        p2 = ctx.enter_context(tc.tile_pool(name="p2", bufs=3))
        p3 = ctx.enter_context(tc.tile_pool(name="p3", bufs=3))
        p1ps = ctx.enter_context(tc.tile_pool(name="p1ps", bufs=1, space="PSUM"))
        work = ctx.enter_context(tc.tile_pool(name="work", bufs=4, space="PSUM"))
        outTp = ctx.enter_context(tc.tile_pool(name="outTp", bufs=1, space="PSUM"))
        sumsp = ctx.enter_context(tc.tile_pool(name="sumsp", bufs=1, space="PSUM"))

        ones = persist.tile([128, 128], DT)
        ident = persist.tile([128, 128], DT)
        nc.sync.dma_start(out=ones, in_=cst[0])
        nc.sync.dma_start(out=ident, in_=cst[1])
        maskt_sb = persist.tile([128, 128], DT)
        nc.sync.dma_start(out=maskt_sb, in_=maskt[:, :])
        bq_sb = persist.tile([1, 768], DT)
        nc.sync.dma_start(out=bq_sb, in_=bqkv[:, :])
        # [d, seg, t]: segs 0..3 = q heads, seg 4 = k
        qkT = persist.tile([128, 5, T], DT)
        vbuf = persist.tile([128, TB, 128], DT)   # [t-in-block, j, d]
        attT = persist.tile([128, HQ, T], DT)     # [d, head, t]
        wqkv_sb = persist.tile([128, CB, 768], DT)
        wo_sb = persist.tile([128, HQ, C], DT)

        def load_wqkv():
            for cc in range(CB):
                nc.sync.dma_start(out=wqkv_sb[:, cc, :], in_=wqkv[cc])

        def load_wo():
            for h in range(HQ):
                nc.sync.dma_start(out=wo_sb[:, h, :], in_=wo[h])

        def p1(s, first=False):
            for tt in range(4 * s, 4 * s + 4):
                xtall = p1x.tile([128, CB, 128], DT, tag="xt")
                for ch in range(4):
                    nc.sync.dma_start(
                        out=xtall[:, 4 * ch:4 * ch + 4, :],
                        in_=xt[tt, :, 512 * ch:512 * (ch + 1)].rearrange(
                            "p (c t) -> p c t", c=4, t=128))
                trig_sb = p1x.tile([128, 8, 64], DT, tag="trig")
                nc.sync.dma_start(out=trig_sb, in_=trig[tt])
                if first and tt == 0:
                    load_wqkv()

                qkv_ps = p1ps.tile([128, 768], F32, tag="qkv")
                for cc in range(CB):
                    nc.tensor.matmul(qkv_ps[:, 0:512], xtall[:, cc, :],
                                     wqkv_sb[:, cc, 0:512],
                                     start=(cc == 0), stop=False)
                    nc.tensor.matmul(qkv_ps[:, 512:768], xtall[:, cc, :],
                                     wqkv_sb[:, cc, 512:768],
                                     start=(cc == 0), stop=False)
                nc.tensor.matmul(qkv_ps[:, 0:512], ones[0:1, :],
                                 bq_sb[0:1, 0:512], start=False, stop=True)
                nc.tensor.matmul(qkv_ps[:, 512:768], ones[0:1, :],
                                 bq_sb[0:1, 512:768], start=False, stop=True)

                # evacuate PSUM fast: q/k raw to SBUF f32, v to vbuf bf16
                qsb = p1s.tile([128, 640], F32, tag="qsb")
                nc.scalar.activation(out=qsb[:, 0:384], in_=qkv_ps[:, 0:384],
                                     func=AF.Copy, scale=1.0, bias=0.0)
                nc.vector.tensor_copy(qsb[:, 384:640], qkv_ps[:, 384:640])
                nc.vector.tensor_copy(vbuf[:, tt, :], qkv_ps[:, 640:768])

                # RMSNorm stats (5 segs: 4 q heads + k)
                ssq = p1s.tile([128, 8], F32, tag="ssq")
                for sg in range(5):
                    sqs = p1s.tile([128, 128], F32, tag="sqs")
                    nc.scalar.activation(out=sqs, in_=qsb[:, sg * 128:(sg + 1) * 128],
                                         func=AF.Square, accum_out=ssq[:, sg:sg + 1])
                x5 = p1s.tile([128, 8], F32, tag="x5")
                nc.scalar.activation(out=x5[:, 0:5], in_=ssq[:, 0:5],
                                     func=AF.Copy, scale=1.0 / D, bias=EPS)
                sq5 = p1s.tile([128, 8], F32, tag="sq5")
                nc.scalar.activation(out=sq5[:, 0:5], in_=x5[:, 0:5], func=AF.Sqrt)
                rstd = p1s.tile([128, 8], F32, tag="rstd")
                nc.vector.reciprocal(out=rstd[:, 0:5], in_=sq5[:, 0:5])

                # RoPE with rstd folded in post-combine; qn_w/kn_w are in trig
                qr = p1s.tile([128, 640], DT, tag="qr")

                def rope(seg0, nseg, toff):
                    src = qsb[:, seg0 * 128:(seg0 + nseg) * 128]
                    dst = qr[:, seg0 * 128:(seg0 + nseg) * 128]
                    sev = src.rearrange("p (h j t) -> p h t j", t=2, j=64)
                    dev = dst.rearrange("p (h j t) -> p h t j", t=2, j=64)
                    qe, qo = sev[:, :, 0, :], sev[:, :, 1, :]
                    re, ro = dev[:, :, 0, :], dev[:, :, 1, :]
                    ce = bc(trig_sb[:, toff + 0, :], nseg)
                    so = bc(trig_sb[:, toff + 1, :], nseg)
                    se = bc(trig_sb[:, toff + 2, :], nseg)
                    co = bc(trig_sb[:, toff + 3, :], nseg)
                    rsb = bass.AP(tensor=rstd.tensor, offset=rstd.offset + seg0,
                                  ap=[list(rstd.ap[0]), [1, nseg], [0, 64]])
                    ta = p1s.tile([128, nseg, 64], F32, tag=f"ra{toff}")
                    tb = p1s.tile([128, nseg, 64], F32, tag=f"rb{toff}")
                    nc.vector.tensor_mul(ta, qe, ce)
                    nc.gpsimd.tensor_mul(tb, qo, so)
                    tr = p1s.tile([128, nseg, 64], F32, tag=f"rr{toff}")
                    nc.vector.tensor_sub(tr, ta, tb)
                    nc.vector.tensor_mul(re, tr, rsb)
                    tcs = p1s.tile([128, nseg, 64], F32, tag=f"rc{toff}")
                    td = p1s.tile([128, nseg, 64], F32, tag=f"rd{toff}")
                    nc.gpsimd.tensor_mul(tcs, qe, se)
                    nc.vector.tensor_mul(td, qo, co)
                    to = p1s.tile([128, nseg, 64], F32, tag=f"ro{toff}")
                    nc.gpsimd.tensor_add(to, tcs, td)
                    nc.gpsimd.tensor_mul(ro, to, rsb)

                rope(0, 4, 0)   # q heads, tables 0..3
                rope(4, 1, 4)   # k, tables 4..7

                # transpose to [d, t] layout
                for sg in range(5):
                    tp = work.tile([128, 512], DT, tag="w")
                    nc.tensor.transpose(tp[:, 0:128], qr[:, sg * 128:(sg + 1) * 128],
                                        ident)
                    dst = qkT[:, sg, tt * 128:(tt + 1) * 128]
                    if sg < 3:
                        nc.scalar.activation(out=dst, in_=tp[:, 0:128],
                                             func=AF.Copy, scale=1.0, bias=0.0)
                    else:
                        nc.vector.tensor_copy(dst, tp[:, 0:128])

        def p2f(s):
            jmax = 4 * s + 3
            q0 = 512 * s
            for h in range(HQ):
                outT = outTp.tile([128, 512], F32, tag="outT")
                sums = sumsp.tile([128, 512], F32, tag="sums")
                for j in range(jmax + 1):
                    qlo = max(q0, j * 128)
                    w = q0 + 512 - qlo
                    poff = qlo - q0
                    diag = qlo == j * 128
                    sc = work.tile([128, 512], F32, tag="w")
                    nc.tensor.matmul(sc[:, 0:w], qkT[:, 4, j * 128:(j + 1) * 128],
                                     qkT[:, h, qlo:q0 + 512], start=True,
                                     stop=not diag)
                    if diag:
                        # additive causal mask folded in as a PE matmul:
                        # sc[:, 0:128] += ident.T @ maskt = maskt
                        nc.tensor.matmul(sc[:, 0:128], ident, maskt_sb,
                                         start=False, stop=True)
                    pT = p2.tile([128, 512], DT, tag="pT")
                    nc.scalar.activation(out=pT[:, 0:w], in_=sc[:, 0:w],
                                         func=AF.Exp, scale=SCALE)
                    nc.tensor.matmul(outT[:, poff:poff + w], vbuf[:, j, :],
                                     pT[:, 0:w], start=(j == 0), stop=(j == jmax),
                                     skip_group_check=True)
                    nc.tensor.matmul(sums[:, poff:poff + w], ones, pT[:, 0:w],
                                     start=(j == 0), stop=(j == jmax),
                                     skip_group_check=True)
                inv = p2.tile([128, 512], F32, tag="inv")
                nc.vector.reciprocal_approx_fast(out=inv, in_=sums)
                nc.vector.tensor_mul(attT[:, h, q0:q0 + 512], outT, inv)

        def p3f(s):
            for tt in range(4 * s, 4 * s + 4):
                for c4 in range(4):
                    y_ps = work.tile([128, 512], F32, tag="w")
                    for h in range(HQ):
                        nc.tensor.matmul(y_ps, attT[:, h, tt * 128:(tt + 1) * 128],
                                         wo_sb[:, h, c4 * 512:(c4 + 1) * 512],
                                         start=(h == 0), stop=(h == HQ - 1))
                    y_sb = p3.tile([128, 512], DT, tag="ysb")
                    if c4 % 2 == 0:
                        nc.vector.tensor_copy(y_sb, y_ps)
                    else:
                        nc.scalar.activation(out=y_sb, in_=y_ps, func=AF.Copy,
                                             scale=1.0, bias=0.0)
                    nc.sync.dma_start(
                        out=y[tt * 128:(tt + 1) * 128, c4 * 512:(c4 + 1) * 512],
                        in_=y_sb)

        p1(0, first=True)
        load_wo()
        p1(1)
        for s in range(4):
            p2f(s)
            p3f(s)
            if s + 2 <= 3:
                p1(s + 2)

    nc.compile()
    return nc


def _prep_core_inputs(b, g, x, Wq, bq, Wk, bk, Wv, bv, Wo, bo, qn_w, kn_w,
                      freqs_cos, freqs_sin, mask, dt_name="float32r",
                      impl="base"):
    f32 = np.float32
    if dt_name == "bfloat16":
        import ml_dtypes
        dt_np = ml_dtypes.bfloat16
    else:
        dt_np = np.float32
    xb = np.ascontiguousarray(x[b], dtype=f32)
    # [tt, csub, cc, tcol]: xt[tt][p][cc*128+tc] = x[b][tt*128+tc][cc*128+p]
    xt = np.ascontiguousarray(
        xb.reshape(TB, 128, CB, 128).transpose(0, 3, 2, 1)
    ).reshape(TB, 128, CB * 128)
    wqkv = np.ascontiguousarray(np.concatenate([
        Wq[:, g * 512:(g + 1) * 512],
        Wk[:, g * 128:(g + 1) * 128],
        Wv[:, g * 128:(g + 1) * 128],
    ], axis=1).reshape(CB, 128, 768), dtype=f32)
    bqkv = np.concatenate([
        bq[g * 512:(g + 1) * 512], bk[g * 128:(g + 1) * 128],
        bv[g * 128:(g + 1) * 128],
    ]).reshape(1, 768).astype(f32)
    cos = freqs_cos.astype(f32)
    sin = freqs_sin.astype(f32)
    qe, qo = qn_w[0::2].astype(f32), qn_w[1::2].astype(f32)
    ke, ko = kn_w[0::2].astype(f32), kn_w[1::2].astype(f32)
    # tables: [ce, so, se, co] for q then for k; layout [TB, 128, 8*64]
    tabs = np.stack([cos * qe, sin * qo, sin * qe, cos * qo,
                     cos * ke, sin * ko, sin * ke, cos * ko], axis=1)  # [T, 8, 64]
    trig = np.ascontiguousarray(tabs.reshape(TB, 128, 8 * 64), dtype=f32)
    maskt = np.ascontiguousarray(mask[0, 0, :128, :128].T, dtype=f32)
    cst = np.stack([np.ones((128, 128), f32), np.eye(128, dtype=f32)])
    wo_t = np.ascontiguousarray(
        Wo[g * 512:(g + 1) * 512].reshape(HQ, 128, C), dtype=f32)
    out = {"xt": xt, "wqkv": wqkv, "bqkv": bqkv, "trig": trig,
           "maskt": maskt, "cst": cst, "wo": wo_t}
    if dt_np is not np.float32:
        keys = ("xt", "wqkv", "bqkv", "trig", "cst", "wo", "maskt") \
            if impl == "fused" else ("xt", "wqkv", "bqkv", "trig", "cst", "wo")
        for k in keys:
            out[k] = out[k].astype(dt_np)
    return out


def kernel(x, Wq, bq, Wk, bk, Wv, bv, Wo, bo, qn_w, kn_w,
           freqs_cos, freqs_sin, mask, _trace=False, _trace_kwargs=None):
    from concourse.bass_utils import run_bass_kernel_spmd

    args = (np.asarray(x), np.asarray(Wq), np.asarray(bq), np.asarray(Wk),
            np.asarray(bk), np.asarray(Wv), np.asarray(bv), np.asarray(Wo),
            np.asarray(bo), np.asarray(qn_w), np.asarray(kn_w),
            np.asarray(freqs_cos), np.asarray(freqs_sin), np.asarray(mask))
    bo_np = args[8].astype(np.float32)

    impl = os.environ.get("BASS_ATTN_IMPL", "base")
    dt_name = os.environ.get("BASS_ATTN_DT", "float32r")
    key = (impl, dt_name)
    if key not in _CACHE:
        _CACHE[key] = _build_fused(dt_name) if impl == "fused" else _build(dt_name)
    nc = _CACHE[key]

    in_maps = [_prep_core_inputs(cid // 4, cid % 4, *args, dt_name=dt_name,
                                 impl=impl)
               for cid in range(8)]
    res = run_bass_kernel_spmd(nc, in_maps, core_ids=list(range(8)),
                               trace=_trace, **(_trace_kwargs or {}))
    outs = [np.asarray(res.results[i]["y"], dtype=np.float32) for i in range(8)]
    yfull = np.empty((B, T, C), dtype=np.float32)
    for b in range(B):
        yfull[b] = outs[4 * b] + outs[4 * b + 1] + outs[4 * b + 2] + outs[4 * b + 3]
        yfull[b] += bo_np[None, :]
    if _trace:
        kernel._last_result = res
    return yfull



# revision 19
# speedup vs baseline: 1.6970x; 1.1292x over previous
"""LLaMA causal self-attention (GQA) on 8 Trainium2 NeuronCores.

Sharding: 2-way data-parallel over batch x 4-way tensor-parallel over KV
groups. Core cid handles batch b=cid//4 and KV group g=cid%4 (q heads
4g..4g+3, kv head g). Each core computes a partial output y_partial =
att_heads @ Wo_rows; the host sums the 4 partials per batch and adds bo.

Per-core pipeline (all layouts chosen so matmul contraction is on the
partition dim and softmax needs no transposes):
  P1: QKV projection (x^T chunks stationary), bias via K=1 ones-row
      matmul, RMSNorm (ACT Square+accum, Newton-refined rsqrt), RoPE
      (elementwise, with qn_w/kn_w folded into the trig tables), then
      PE-transpose q/k to [d, t] layout.
  P2: attention computed transposed: scoresT[k, q] = kT_j^T @ qT chunks,
      additive causal mask on diagonal blocks, exp without max
      subtraction (RMS-normed scores are bounded, softmax is shift
      invariant), softmax denominators via an all-ones stationary matmul
      (sums land broadcast across all partitions), PV accumulated in
      PSUM over j, one normalize multiply per (head, half).
  P3: output projection from attT chunks, PSUM -> SBUF -> DRAM.
"""

import os
from contextlib import ExitStack

import numpy as np

B, T, C = 2, 2048, 2048
H, KV = 16, 4
D = 128
HQ = H // KV        # q heads per core = 4
TB = T // 128       # 16
CB = C // 128       # 16
EPS = 1e-5
SCALE = float(np.float32(1.0) / np.sqrt(np.float32(D)))

_CACHE = {}


def _build(dt_name):
    import concourse.bass as bass
    import concourse.bacc as bacc
    from concourse import mybir
    from concourse.tile import TileContext

    DT = getattr(mybir.dt, dt_name)
    F32 = mybir.dt.float32
    AF = mybir.ActivationFunctionType
    ALU = mybir.AluOpType

    nc = bacc.Bacc(None, target_bir_lowering=False)
    xt = nc.dram_tensor("xt", [TB, 128, CB * 128], DT, kind="ExternalInput")
    wqkv = nc.dram_tensor("wqkv", [CB, 128, 768], DT, kind="ExternalInput")
    bqkv = nc.dram_tensor("bqkv", [1, 768], DT, kind="ExternalInput")
    trig = nc.dram_tensor("trig", [TB, 128, 8 * 64], DT, kind="ExternalInput")
    maskt = nc.dram_tensor("maskt", [128, 128], F32, kind="ExternalInput")
    cst = nc.dram_tensor("cst", [2, 128, 128], DT, kind="ExternalInput")
    wo = nc.dram_tensor("wo", [HQ, 128, C], DT, kind="ExternalInput")
    y = nc.dram_tensor("y", [T, C], F32, kind="ExternalOutput")

    def bc4(apv, n):
        # broadcast a [128, 64] AP along a new middle (head) dim of size n
        return bass.AP(tensor=apv.tensor, offset=apv.offset,
                       ap=[list(apv.ap[0]), [0, n], [1, 64]])

    with TileContext(nc) as tc, ExitStack() as ctx:
        persist = ctx.enter_context(tc.tile_pool(name="persist", bufs=1))
        ones = persist.tile([128, 128], DT)
        ident = persist.tile([128, 128], DT)
        nc.sync.dma_start(out=ones, in_=cst[0])
        nc.sync.dma_start(out=ident, in_=cst[1])
        maskt_sb = persist.tile([128, 128], F32)
        nc.sync.dma_start(out=maskt_sb, in_=maskt[:, :])
        bq_sb = persist.tile([1, 768], DT)
        nc.sync.dma_start(out=bq_sb, in_=bqkv[:, :])
        # [d, seg, t]: segs 0..3 = q heads, seg 4 = k
        qkT = persist.tile([128, 5, T], DT)
        vbuf = persist.tile([128, TB, 128], DT)   # [t-in-block, j, d]
        attT = persist.tile([128, HQ, T], DT)     # [d, head, t]
        # ---------------- P1: QKV + RMSNorm + RoPE + transpose ----------
        with tc.tile_pool(name="p1", bufs=2) as p1, \
             tc.tile_pool(name="p1x", bufs=2) as p1x, \
             tc.tile_pool(name="p1s", bufs=3) as p1s, \
             tc.tile_pool(name="p1w", bufs=1) as p1w, \
             tc.tile_pool(name="p1ps", bufs=2, space="PSUM") as p1ps, \
             tc.tile_pool(name="p1tp", bufs=2, space="PSUM") as p1tp:
            wqkv_sb = p1w.tile([128, CB, 768], DT)
            for cc in range(CB):
                nc.sync.dma_start(out=wqkv_sb[:, cc, :], in_=wqkv[cc])
            for tt in range(TB):
                xtall = p1x.tile([128, CB, 128], DT, tag="xt")
                nc.sync.dma_start(out=xtall, in_=xt[tt])
                trig_sb = p1x.tile([128, 8, 64], DT, tag="trig")
                nc.sync.dma_start(out=trig_sb, in_=trig[tt])

                qkv_ps = p1ps.tile([128, 768], F32, tag="qkv")
                for cc in range(CB):
                    nc.tensor.matmul(qkv_ps[:, 0:512], xtall[:, cc, :],
                                     wqkv_sb[:, cc, 0:512],
                                     start=(cc == 0), stop=False)
                    nc.tensor.matmul(qkv_ps[:, 512:768], xtall[:, cc, :],
                                     wqkv_sb[:, cc, 512:768],
                                     start=(cc == 0), stop=False)
                nc.tensor.matmul(qkv_ps[:, 0:512], ones[0:1, :],
                                 bq_sb[0:1, 0:512], start=False, stop=True)
                nc.tensor.matmul(qkv_ps[:, 512:768], ones[0:1, :],
                                 bq_sb[0:1, 512:768], start=False, stop=True)

                # RMSNorm stats for 4 q heads + k
                ssq = p1s.tile([128, 8], F32, tag="ssq")
                for s in range(5):
                    sqs = p1s.tile([128, 128], F32, tag="sqs")
                    nc.scalar.activation(out=sqs, in_=qkv_ps[:, s * 128:(s + 1) * 128],
                                         func=AF.Square, accum_out=ssq[:, s:s + 1])
                x5 = p1s.tile([128, 8], F32, tag="x5")
                nc.scalar.activation(out=x5[:, 0:5], in_=ssq[:, 0:5],
                                     func=AF.Copy, scale=1.0 / D, bias=EPS)
                sq5 = p1s.tile([128, 8], F32, tag="sq5")
                nc.scalar.activation(out=sq5[:, 0:5], in_=x5[:, 0:5], func=AF.Sqrt)
                r0 = p1s.tile([128, 8], F32, tag="r0")
                nc.vector.reciprocal(out=r0[:, 0:5], in_=sq5[:, 0:5])
                # one Newton step: rstd = r0 * (1.5 - 0.5 * x * r0^2)
                t1 = p1s.tile([128, 8], F32, tag="t1")
                nc.vector.tensor_mul(t1[:, 0:5], r0[:, 0:5], r0[:, 0:5])
                t2 = p1s.tile([128, 8], F32, tag="t2")
                nc.vector.scalar_tensor_tensor(out=t2[:, 0:5], in0=t1[:, 0:5],
                                               scalar=-0.5, in1=x5[:, 0:5],
                                               op0=ALU.mult, op1=ALU.mult)
                t3 = p1s.tile([128, 8], F32, tag="t3")
                nc.vector.tensor_scalar_add(t3[:, 0:5], t2[:, 0:5], 1.5)
                rstd = p1s.tile([128, 8], F32, tag="rstd")
                nc.vector.tensor_mul(rstd[:, 0:5], r0[:, 0:5], t3[:, 0:5])

                # normalize q/k (x * rstd), copy v
                qn = p1.tile([128, 640], F32, tag="qn")
                for s in range(5):
                    nc.scalar.activation(out=qn[:, s * 128:(s + 1) * 128],
                                         in_=qkv_ps[:, s * 128:(s + 1) * 128],
                                         func=AF.Copy, scale=rstd[:, s:s + 1], bias=0.0)
                nc.scalar.activation(out=vbuf[:, tt, :], in_=qkv_ps[:, 640:768],
                                     func=AF.Copy, scale=1.0, bias=0.0)

                # RoPE (norm weights folded into trig tables host-side)
                qr = p1.tile([128, 640], DT, tag="qr")

                def rope(seg0, nseg, toff):
                    src = qn[:, seg0 * 128:(seg0 + nseg) * 128]
                    dst = qr[:, seg0 * 128:(seg0 + nseg) * 128]
                    sev = src.rearrange("p (h j t) -> p h t j", t=2, j=64)
                    dev = dst.rearrange("p (h j t) -> p h t j", t=2, j=64)
                    qe, qo = sev[:, :, 0, :], sev[:, :, 1, :]
                    re, ro = dev[:, :, 0, :], dev[:, :, 1, :]
                    ce = bc4(trig_sb[:, toff + 0, :], nseg)
                    so = bc4(trig_sb[:, toff + 1, :], nseg)
                    se = bc4(trig_sb[:, toff + 2, :], nseg)
                    co = bc4(trig_sb[:, toff + 3, :], nseg)
                    ta = p1s.tile([128, nseg, 64], F32, tag=f"ra{toff}")
                    tb = p1s.tile([128, nseg, 64], F32, tag=f"rb{toff}")
                    nc.vector.tensor_mul(ta, qe, ce)
                    nc.gpsimd.tensor_mul(tb, qo, so)
                    nc.vector.tensor_sub(re, ta, tb)
                    tc_ = p1s.tile([128, nseg, 64], F32, tag=f"rc{toff}")
                    td = p1s.tile([128, nseg, 64], F32, tag=f"rd{toff}")
                    nc.gpsimd.tensor_mul(tc_, qe, se)
                    nc.vector.tensor_mul(td, qo, co)
                    nc.gpsimd.tensor_add(ro, tc_, td)

                rope(0, 4, 0)   # q heads, tables 0..3
                rope(4, 1, 4)   # k, tables 4..7

                # transpose to [d, t]
                for s in range(5):
                    tp = p1tp.tile([128, 128], DT, tag="tp")
                    nc.tensor.transpose(tp, qr[:, s * 128:(s + 1) * 128], ident)
                    nc.scalar.activation(out=qkT[:, s, tt * 128:(tt + 1) * 128],
                                         in_=tp, func=AF.Copy, scale=1.0, bias=0.0)

        # ---------------- P2: attention (transposed scores) -------------
        with tc.tile_pool(name="p2", bufs=2) as p2, \
             tc.tile_pool(name="p2acc", bufs=1, space="PSUM") as p2acc, \
             tc.tile_pool(name="p2sc", bufs=2, space="PSUM") as p2sc:
            for h in range(HQ):
                for half in range(2):
                    q0 = half * 1024
                    q1 = q0 + 1024
                    jmax = 7 if half == 0 else 15
                    outT = p2acc.tile([128, 1024], F32, tag="outT")
                    sums = p2acc.tile([128, 1024], F32, tag="sums")
                    for j in range(jmax + 1):
                        qlo = max(q0, j * 128)
                        pT = p2.tile([128, 1024], DT, tag="pT")
                        kTj = qkT[:, 4, j * 128:(j + 1) * 128]
                        for ci in range(2):
                            begin = max(qlo, q0 + 512 * ci)
                            end = q0 + 512 * (ci + 1)
                            if begin >= end:
                                continue
                            w = end - begin
                            sc = p2sc.tile([128, 512], F32, tag="sc")
                            nc.tensor.matmul(sc[:, 0:w], kTj,
                                             qkT[:, h, begin:end],
                                             start=True, stop=True)
                            if begin == j * 128:
                                nc.vector.tensor_add(sc[:, 0:128],
                                                     sc[:, 0:128], maskt_sb)
                            poff = begin - qlo
                            nc.scalar.activation(out=pT[:, poff:poff + w],
                                                 in_=sc[:, 0:w], func=AF.Exp,
                                                 scale=SCALE)
                            nc.tensor.matmul(outT[:, begin - q0:begin - q0 + w],
                                             vbuf[:, j, :], pT[:, poff:poff + w],
                                             start=(j == 0), stop=(j == jmax),
                                             skip_group_check=True)
                            nc.tensor.matmul(sums[:, begin - q0:begin - q0 + w],
                                             ones, pT[:, poff:poff + w],
                                             start=(j == 0), stop=(j == jmax),
                                             skip_group_check=True)
                    inv = p2.tile([128, 1024], F32, tag="inv")
                    nc.vector.reciprocal(out=inv, in_=sums)
                    nc.vector.tensor_mul(attT[:, h, q0:q1], outT, inv)

        # ---------------- P3: output projection --------------------------
        with tc.tile_pool(name="p3", bufs=2) as p3, \
             tc.tile_pool(name="p3w", bufs=1) as p3w, \
             tc.tile_pool(name="p3ps", bufs=2, space="PSUM") as p3ps:
            wo_sb = p3w.tile([128, HQ, C], DT)
            for h in range(HQ):
                nc.sync.dma_start(out=wo_sb[:, h, :], in_=wo[h])
            for tt in range(TB):
                y_ps = p3ps.tile([128, 2048], F32, tag="y")
                for h in range(HQ):
                    for c4 in range(4):
                        nc.tensor.matmul(y_ps[:, c4 * 512:(c4 + 1) * 512],
                                         attT[:, h, tt * 128:(tt + 1) * 128],
                                         wo_sb[:, h, c4 * 512:(c4 + 1) * 512],
                                         start=(h == 0), stop=(h == HQ - 1))
                y_sb = p3.tile([128, 2048], F32, tag="ysb")
                nc.scalar.activation(out=y_sb[:, 0:1024], in_=y_ps[:, 0:1024],
                                     func=AF.Copy, scale=1.0, bias=0.0)
                nc.vector.tensor_copy(y_sb[:, 1024:2048], y_ps[:, 1024:2048])
                nc.sync.dma_start(out=y[tt * 128:(tt + 1) * 128, :], in_=y_sb)

    nc.compile()
    return nc


def _build_fused(dt_name):
    """Fused software pipeline: P1 (QKV+RMSNorm+RoPE+transpose), P2
    (attention) and P3 (output proj) interleaved over 512-row q-stripes
    so the tensor engine stays dense and warm.  Emission order:
    P1(0) P1(1) [P2(s) P3(s) P1(s+2)] for s=0..3.

    PSUM budget (8 banks): qkv accumulator [128,768] x1 = 2, shared
    work pool [128,512] x4 = 4 (scores / transposes / y chunks),
    outT x1 = 1, sums x1 = 1.
    """
    import concourse.bass as bass
    import concourse.bacc as bacc
    from concourse import mybir
    from concourse.tile import TileContext

    DT = getattr(mybir.dt, dt_name)
    F32 = mybir.dt.float32
    AF = mybir.ActivationFunctionType
    ALU = mybir.AluOpType

    nc = bacc.Bacc(None, target_bir_lowering=False)
    xt = nc.dram_tensor("xt", [TB, 128, CB * 128], DT, kind="ExternalInput")
    wqkv = nc.dram_tensor("wqkv", [CB, 128, 768], DT, kind="ExternalInput")
    bqkv = nc.dram_tensor("bqkv", [1, 768], DT, kind="ExternalInput")
    trig = nc.dram_tensor("trig", [TB, 128, 8 * 64], DT, kind="ExternalInput")
    maskt = nc.dram_tensor("maskt", [128, 128], DT, kind="ExternalInput")
    cst = nc.dram_tensor("cst", [2, 128, 128], DT, kind="ExternalInput")
    wo = nc.dram_tensor("wo", [HQ, 128, C], DT, kind="ExternalInput")
    y = nc.dram_tensor("y", [T, C], DT, kind="ExternalOutput")

    def bc(apv, n, w=64):
        # broadcast a [128, w] AP along a new middle dim of size n
        return bass.AP(tensor=apv.tensor, offset=apv.offset,
                       ap=[list(apv.ap[0]), [0, n], [1, w]])

    with TileContext(nc) as tc, ExitStack() as ctx:
        persist = ctx.enter_context(tc.tile_pool(name="persist", bufs=1))
        p1x = ctx.enter_context(tc.tile_pool(name="p1x", bufs=3))
        p1s = ctx.enter_context(tc.tile_pool(name="p1s", bufs=3))
        p2 = ctx.enter_context(tc.tile_pool(name="p2", bufs=3))
        p3 = ctx.enter_context(tc.tile_pool(name="p3", bufs=3))
        qps = ctx.enter_context(tc.tile_pool(name="qps", bufs=1, space="PSUM"))
        kvps = ctx.enter_context(tc.tile_pool(name="kvps", bufs=1, space="PSUM"))
        work = ctx.enter_context(tc.tile_pool(name="work", bufs=4, space="PSUM"))
        outTp = ctx.enter_context(tc.tile_pool(name="outTp", bufs=1, space="PSUM"))
        sumsp = ctx.enter_context(tc.tile_pool(name="sumsp", bufs=1, space="PSUM"))

        ones = persist.tile([128, 128], DT)
        ident = persist.tile([128, 128], DT)
        nc.sync.dma_start(out=ones, in_=cst[0])
        nc.sync.dma_start(out=ident, in_=cst[1])
        maskt_sb = persist.tile([128, 128], DT)
        nc.sync.dma_start(out=maskt_sb, in_=maskt[:, :])
        bq_sb = persist.tile([1, 768], DT)
        nc.sync.dma_start(out=bq_sb, in_=bqkv[:, :])
        # [d, seg, t]: segs 0..3 = q heads, seg 4 = k
        qkT = persist.tile([128, 5, T], DT)
        vbuf = persist.tile([128, TB, 128], DT)   # [t-in-block, j, d]
        attT = persist.tile([128, HQ, T], DT)     # [d, head, t]
        wqkv_sb = persist.tile([128, CB, 768], DT)
        wo_sb = persist.tile([128, HQ, C], DT)

        def load_wqkv():
            for cc in range(CB):
                nc.sync.dma_start(out=wqkv_sb[:, cc, :], in_=wqkv[cc])

        def load_wo():
            for h in range(HQ):
                nc.sync.dma_start(out=wo_sb[:, h, :], in_=wo[h])

        def p1(s, first=False):
            for tt in range(4 * s, 4 * s + 4):
                xtall = p1x.tile([128, CB, 128], DT, tag="xt")
                for ch in range(4):
                    nc.sync.dma_start(
                        out=xtall[:, 4 * ch:4 * ch + 4, :],
                        in_=xt[tt, :, 512 * ch:512 * (ch + 1)].rearrange(
                            "p (c t) -> p c t", c=4, t=128))
                trig_sb = p1x.tile([128, 8, 64], DT, tag="trig")
                nc.sync.dma_start(out=trig_sb, in_=trig[tt])
                if first and tt == 0:
                    load_wqkv()

                # q heads accumulate in one bank, k/v in another; the kv
                # matmul loop covers q's PSUM evacuation and vice versa
                q_ps = qps.tile([128, 512], F32, tag="q")
                for cc in range(CB):
                    nc.tensor.matmul(q_ps, xtall[:, cc, :],
                                     wqkv_sb[:, cc, 0:512],
                                     start=(cc == 0), stop=False)
                nc.tensor.matmul(q_ps, ones[0:1, :],
                                 bq_sb[0:1, 0:512], start=False, stop=True)
                kv_ps = kvps.tile([128, 256], F32, tag="kv")
                for cc in range(CB):
                    nc.tensor.matmul(kv_ps, xtall[:, cc, :],
                                     wqkv_sb[:, cc, 512:768],
                                     start=(cc == 0), stop=False)
                nc.tensor.matmul(kv_ps, ones[0:1, :],
                                 bq_sb[0:1, 512:768], start=False, stop=True)

                # evacuate PSUM fast (DVE-heavy; ACT is exp-loaded)
                qsb = p1s.tile([128, 640], F32, tag="qsb")
                nc.vector.tensor_copy(qsb[:, 0:512], q_ps)
                nc.scalar.activation(out=qsb[:, 512:640], in_=kv_ps[:, 0:128],
                                     func=AF.Copy, scale=1.0, bias=0.0)
                nc.vector.tensor_copy(vbuf[:, tt, :], kv_ps[:, 128:256])

                # RMSNorm stats (5 segs: 4 q heads + k)
                ssq = p1s.tile([128, 8], F32, tag="ssq")
                for sg in range(5):
                    sqs = p1s.tile([128, 128], F32, tag="sqs")
                    nc.scalar.activation(out=sqs,
                                         in_=qsb[:, sg * 128:(sg + 1) * 128],
                                         func=AF.Square,
                                         accum_out=ssq[:, sg:sg + 1])
                x5 = p1s.tile([128, 8], F32, tag="x5")
                nc.scalar.activation(out=x5[:, 0:5], in_=ssq[:, 0:5],
                                     func=AF.Copy, scale=1.0 / D, bias=EPS)
                sq5 = p1s.tile([128, 8], F32, tag="sq5")
                nc.scalar.activation(out=sq5[:, 0:5], in_=x5[:, 0:5], func=AF.Sqrt)
                rstd = p1s.tile([128, 8], F32, tag="rstd")
                nc.vector.reciprocal(out=rstd[:, 0:5], in_=sq5[:, 0:5])

                # RoPE with rstd folded in post-combine; qn_w/kn_w are in trig
                qr = p1s.tile([128, 640], DT, tag="qr")

                def rope(seg0, nseg, toff):
                    src = qsb[:, seg0 * 128:(seg0 + nseg) * 128]
                    dst = qr[:, seg0 * 128:(seg0 + nseg) * 128]
                    sev = src.rearrange("p (h j t) -> p h t j", t=2, j=64)
                    dev = dst.rearrange("p (h j t) -> p h t j", t=2, j=64)
                    qe, qo = sev[:, :, 0, :], sev[:, :, 1, :]
                    re, ro = dev[:, :, 0, :], dev[:, :, 1, :]
                    ce = bc(trig_sb[:, toff + 0, :], nseg)
                    so = bc(trig_sb[:, toff + 1, :], nseg)
                    se = bc(trig_sb[:, toff + 2, :], nseg)
                    co = bc(trig_sb[:, toff + 3, :], nseg)
                    rsb = bass.AP(tensor=rstd.tensor, offset=rstd.offset + seg0,
                                  ap=[list(rstd.ap[0]), [1, nseg], [0, 64]])
                    ta = p1s.tile([128, nseg, 64], F32, tag=f"ra{toff}")
                    tb = p1s.tile([128, nseg, 64], F32, tag=f"rb{toff}")
                    nc.vector.tensor_mul(ta, qe, ce)
                    nc.gpsimd.tensor_mul(tb, qo, so)
                    tr = p1s.tile([128, nseg, 64], F32, tag=f"rr{toff}")
                    nc.vector.tensor_sub(tr, ta, tb)
                    nc.vector.tensor_mul(re, tr, rsb)
                    tcs = p1s.tile([128, nseg, 64], F32, tag=f"rc{toff}")
                    td = p1s.tile([128, nseg, 64], F32, tag=f"rd{toff}")
                    nc.gpsimd.tensor_mul(tcs, qe, se)
                    nc.vector.tensor_mul(td, qo, co)
                    to = p1s.tile([128, nseg, 64], F32, tag=f"ro{toff}")
                    nc.gpsimd.tensor_add(to, tcs, td)
                    nc.gpsimd.tensor_mul(ro, to, rsb)

                rope(0, 4, 0)   # q heads, tables 0..3
                rope(4, 1, 4)   # k, tables 4..7

                # transpose to [d, t] layout
                for sg in range(5):
                    tp = work.tile([128, 512], DT, tag="w")
                    nc.tensor.transpose(tp[:, 0:128], qr[:, sg * 128:(sg + 1) * 128],
                                        ident)
                    dst = qkT[:, sg, tt * 128:(tt + 1) * 128]
                    if sg < 3:
                        nc.scalar.activation(out=dst, in_=tp[:, 0:128],
                                             func=AF.Copy, scale=1.0, bias=0.0)
                    else:
                        nc.vector.tensor_copy(dst, tp[:, 0:128])

        def p2f(s):
            jmax = 4 * s + 3
            q0 = 512 * s
            for h in range(HQ):
                outT = outTp.tile([128, 512], F32, tag="outT")
                sums = sumsp.tile([128, 512], F32, tag="sums")
                for j in range(jmax + 1):
                    qlo = max(q0, j * 128)
                    w = q0 + 512 - qlo
                    poff = qlo - q0
                    diag = qlo == j * 128
                    sc = work.tile([128, 512], F32, tag="w")
                    nc.tensor.matmul(sc[:, 0:w], qkT[:, 4, j * 128:(j + 1) * 128],
                                     qkT[:, h, qlo:q0 + 512], start=True,
                                     stop=not diag)
                    if diag:
                        # additive causal mask folded in as a PE matmul:
                        # sc[:, 0:128] += ident.T @ maskt = maskt
                        nc.tensor.matmul(sc[:, 0:128], ident, maskt_sb,
                                         start=False, stop=True)
                    pT = p2.tile([128, 512], DT, tag="pT")
                    nc.scalar.activation(out=pT[:, 0:w], in_=sc[:, 0:w],
                                         func=AF.Exp, scale=SCALE)
                    nc.tensor.matmul(outT[:, poff:poff + w], vbuf[:, j, :],
                                     pT[:, 0:w], start=(j == 0), stop=(j == jmax),
                                     skip_group_check=True)
                    nc.tensor.matmul(sums[:, poff:poff + w], ones, pT[:, 0:w],
                                     start=(j == 0), stop=(j == jmax),
                                     skip_group_check=True)
                inv = p2.tile([128, 512], F32, tag="inv")
                nc.vector.reciprocal_approx_fast(out=inv, in_=sums)
                nc.vector.tensor_mul(attT[:, h, q0:q0 + 512], outT, inv)

        def p3f(s):
            for tt in range(4 * s, 4 * s + 4):
                for c4 in range(4):
                    y_ps = work.tile([128, 512], F32, tag="w")
                    for h in range(HQ):
                        nc.tensor.matmul(y_ps, attT[:, h, tt * 128:(tt + 1) * 128],
                                         wo_sb[:, h, c4 * 512:(c4 + 1) * 512],
                                         start=(h == 0), stop=(h == HQ - 1))
                    y_sb = p3.tile([128, 512], DT, tag="ysb")
                    if c4 % 2 == 0:
                        nc.vector.tensor_copy(y_sb, y_ps)
                    else:
                        nc.scalar.activation(out=y_sb, in_=y_ps, func=AF.Copy,
                                             scale=1.0, bias=0.0)
                    nc.sync.dma_start(
                        out=y[tt * 128:(tt + 1) * 128, c4 * 512:(c4 + 1) * 512],
                        in_=y_sb)

        p1(0, first=True)
        load_wo()
        p1(1)
        for s in range(4):
            p2f(s)
            p3f(s)
            if s + 2 <= 3:
                p1(s + 2)

    nc.compile()
    return nc


def _prep_core_inputs(b, g, x, Wq, bq, Wk, bk, Wv, bv, Wo, bo, qn_w, kn_w,
                      freqs_cos, freqs_sin, mask, dt_name="float32r",
                      impl="base"):
    f32 = np.float32
    if dt_name == "bfloat16":
        import ml_dtypes
        dt_np = ml_dtypes.bfloat16
    else:
        dt_np = np.float32
    xb = np.ascontiguousarray(x[b], dtype=f32)
    # [tt, csub, cc, tcol]: xt[tt][p][cc*128+tc] = x[b][tt*128+tc][cc*128+p]
    xt = np.ascontiguousarray(
        xb.reshape(TB, 128, CB, 128).transpose(0, 3, 2, 1)
    ).reshape(TB, 128, CB * 128)
    wqkv = np.ascontiguousarray(np.concatenate([
        Wq[:, g * 512:(g + 1) * 512],
        Wk[:, g * 128:(g + 1) * 128],
        Wv[:, g * 128:(g + 1) * 128],
    ], axis=1).reshape(CB, 128, 768), dtype=f32)
    bqkv = np.concatenate([
        bq[g * 512:(g + 1) * 512], bk[g * 128:(g + 1) * 128],
        bv[g * 128:(g + 1) * 128],
    ]).reshape(1, 768).astype(f32)
    cos = freqs_cos.astype(f32)
    sin = freqs_sin.astype(f32)
    qe, qo = qn_w[0::2].astype(f32), qn_w[1::2].astype(f32)
    ke, ko = kn_w[0::2].astype(f32), kn_w[1::2].astype(f32)
    # tables: [ce, so, se, co] for q then for k; layout [TB, 128, 8*64]
    tabs = np.stack([cos * qe, sin * qo, sin * qe, cos * qo,
                     cos * ke, sin * ko, sin * ke, cos * ko], axis=1)  # [T, 8, 64]
    trig = np.ascontiguousarray(tabs.reshape(TB, 128, 8 * 64), dtype=f32)
    maskt = np.ascontiguousarray(mask[0, 0, :128, :128].T, dtype=f32)
    cst = np.stack([np.ones((128, 128), f32), np.eye(128, dtype=f32)])
    wo_t = np.ascontiguousarray(
        Wo[g * 512:(g + 1) * 512].reshape(HQ, 128, C), dtype=f32)
    out = {"xt": xt, "wqkv": wqkv, "bqkv": bqkv, "trig": trig,
           "maskt": maskt, "cst": cst, "wo": wo_t}
    if dt_np is not np.float32:
        keys = ("xt", "wqkv", "bqkv", "trig", "cst", "wo", "maskt") \
            if impl == "fused" else ("xt", "wqkv", "bqkv", "trig", "cst", "wo")
        for k in keys:
            out[k] = out[k].astype(dt_np)
    return out


def kernel(x, Wq, bq, Wk, bk, Wv, bv, Wo, bo, qn_w, kn_w,
           freqs_cos, freqs_sin, mask, _trace=False, _trace_kwargs=None):
    from concourse.bass_utils import run_bass_kernel_spmd

    args = (np.asarray(x), np.asarray(Wq), np.asarray(bq), np.asarray(Wk),
            np.asarray(bk), np.asarray(Wv), np.asarray(bv), np.asarray(Wo),
            np.asarray(bo), np.asarray(qn_w), np.asarray(kn_w),
            np.asarray(freqs_cos), np.asarray(freqs_sin), np.asarray(mask))
    bo_np = args[8].astype(np.float32)

    impl = os.environ.get("BASS_ATTN_IMPL", "base")
    dt_name = os.environ.get("BASS_ATTN_DT", "float32r")
    key = (impl, dt_name)
    if key not in _CACHE:
        _CACHE[key] = _build_fused(dt_name) if impl == "fused" else _build(dt_name)
    nc = _CACHE[key]

    in_maps = [_prep_core_inputs(cid // 4, cid % 4, *args, dt_name=dt_name,
                                 impl=impl)
               for cid in range(8)]
    res = run_bass_kernel_spmd(nc, in_maps, core_ids=list(range(8)),
                               trace=_trace, **(_trace_kwargs or {}))
    outs = [np.asarray(res.results[i]["y"], dtype=np.float32) for i in range(8)]
    yfull = np.empty((B, T, C), dtype=np.float32)
    for b in range(B):
        yfull[b] = outs[4 * b] + outs[4 * b + 1] + outs[4 * b + 2] + outs[4 * b + 3]
        yfull[b] += bo_np[None, :]
    if _trace:
        kernel._last_result = res
    return yfull

